# revision 7
# baseline (speedup 1.0000x reference)
"""Trainium2 Bass kernel: greedy bbox-matching loss (nn_BboxLoss).

Full computation: L[t,p] = pairwise bbox loss (IoU / MSE mix), then greedy
per-target argmin over still-available preds, mean of selected losses.

Strategy (8 NeuronCores, preds sharded 8 x 1024):
  device: per core, compute negated-loss tiles [128 targets x 1024 preds]
          entirely in SBUF, then top-8 per target via max/max_index.
          - PE matmul (K=6) produces -mse directly in PSUM (coords cross-term
            plus folded -sq_p/4 and -sq_t/4 rows).
          - ACT computes the min(corner) terms via warm-table Relu chains.
          - DVE does the remaining elementwise passes + top-8 extraction.
  host:   merge the 8x8=64 candidates per target, run the (inherently
          sequential, O(T*64)) greedy walk with an exactness safety check;
          rare unsafe rows fall back to a full-row recompute.

Device returns v = iou (overlap) or -mse (no overlap); true loss = 1 - v.
"""
import numpy as np
from contextlib import ExitStack

P_TOTAL = 8192
T = 2048
N_CORES = 8
P_CORE = P_TOTAL // N_CORES   # 1024
NJ = T // 128                 # 16 row tiles of 128 targets
EPS = 1e-7
TOPK = 8

_CACHE = {}


def _build_nc():
    import concourse.bacc as bacc
    import concourse.mybir as mybir
    from concourse.tile import TileContext

    f32 = mybir.dt.float32
    u32 = mybir.dt.uint32
    i32 = mybir.dt.int32
    Alu = mybir.AluOpType
    Act = mybir.ActivationFunctionType

    nc = bacc.Bacc()
    ps_d = nc.dram_tensor("pshard", [1, 5 * P_CORE], f32, kind="ExternalInput")
    hp_d = nc.dram_tensor("hpredT", [6, P_CORE], f32, kind="ExternalInput")
    tsc_d = nc.dram_tensor("tscal", [5, T], f32, kind="ExternalInput")
    tt_d = nc.dram_tensor("tgtT", [6, T], f32, kind="ExternalInput")
    vals_d = nc.dram_tensor("cand_vals", [T, TOPK], f32, kind="ExternalOutput")
    idx_d = nc.dram_tensor("cand_idx", [T, TOPK], u32, kind="ExternalOutput")

    with TileContext(nc) as tc, ExitStack() as ctx:
        const = ctx.enter_context(tc.tile_pool(name="const", bufs=1))
        work = ctx.enter_context(tc.tile_pool(name="work", bufs=3))
        psum = ctx.enter_context(
            tc.tile_pool(name="psum", bufs=4, space="PSUM")
        )

        PS = const.tile([1, 5 * P_CORE], f32)
        HP = const.tile([6, P_CORE], f32)
        TSC = const.tile([128, 5, NJ], f32)
        TT6 = const.tile([6, T], f32)
        PLANES = const.tile([128, 5, P_CORE], f32)
        VALS = const.tile([128, NJ, TOPK], f32)
        IDX = const.tile([128, NJ, TOPK], u32)

        nc.sync.dma_start(PS[:], ps_d[:])
        nc.sync.dma_start(HP[:], hp_d[:])
        nc.sync.dma_start(TSC[:], tsc_d[:].rearrange("q (j p) -> p q j", p=128))
        nc.sync.dma_start(TT6[:], tt_d[:])

        # replicate the five per-pred rows across all 128 partitions (layouts
        # of PS and PLANES match, so one broadcast covers all five planes)
        nc.gpsimd.partition_broadcast(
            PLANES[:].rearrange("p q n -> p (q n)"), PS[0:1, :]
        )

        X0P = PLANES[:, 0, :]
        X1P = PLANES[:, 1, :]
        Y0P = PLANES[:, 2, :]
        Y1P = PLANES[:, 3, :]
        AREAP = PLANES[:, 4, :]

        for j in range(NJ):
            x0t = TSC[:, 0, j : j + 1]
            x1t = TSC[:, 1, j : j + 1]
            y0t = TSC[:, 2, j : j + 1]
            y1t = TSC[:, 3, j : j + 1]
            ate = TSC[:, 4, j : j + 1]   # area_t + EPS

            # nl1 = cross/2 - sq_t/4 - sq_p/4 = -mse, straight out of PE
            nl1 = psum.tile([128, P_CORE], f32, tag="nl1")
            lhsT = TT6[:, j * 128 : (j + 1) * 128]
            nc.tensor.matmul(nl1[:, 0:512], lhsT, HP[:, 0:512], start=True, stop=True)
            nc.tensor.matmul(nl1[:, 512:1024], lhsT, HP[:, 512:1024], start=True, stop=True)

            ox1 = work.tile([128, P_CORE], f32, tag="ox1")
            oy1 = work.tile([128, P_CORE], f32, tag="oy1")
            ndx = work.tile([128, P_CORE], f32, tag="ndx")
            ndy = work.tile([128, P_CORE], f32, tag="ndy")
            inter = work.tile([128, P_CORE], f32, tag="inter")
            nmr = work.tile([128, P_CORE], f32, tag="nmr")
            dneg = work.tile([128, P_CORE], f32, tag="dneg")
            rcp = work.tile([128, P_CORE], f32, tag="rcp")
            fin = work.tile([128, P_CORE], f32, tag="fin")

            # ox1 = min(x1p, x1t) = relu(x1t - relu(x1t - x1p)); warm Relu table
            nc.scalar.activation(ox1[:], X1P, Act.Relu, bias=x1t, scale=-1.0)
            nc.scalar.activation(ox1[:], ox1[:], Act.Relu, bias=x1t, scale=-1.0)
            nc.scalar.activation(oy1[:], Y1P, Act.Relu, bias=y1t, scale=-1.0)
            nc.scalar.activation(oy1[:], oy1[:], Act.Relu, bias=y1t, scale=-1.0)

            # ndx = max(x0p, x0t) - ox1  (= -dx)
            nc.vector.scalar_tensor_tensor(
                ndx[:], X0P, x0t, ox1[:], op0=Alu.max, op1=Alu.subtract
            )
            nc.vector.scalar_tensor_tensor(
                ndy[:], Y0P, y0t, oy1[:], op0=Alu.max, op1=Alu.subtract
            )
            # inter = ndx*ndy (= dx*dy)
            nc.vector.tensor_tensor(inter[:], ndx[:], ndy[:], op=Alu.mult)
            # nmr = relu(max(ndx, ndy)): bit pattern nonzero <=> no overlap
            nc.vector.scalar_tensor_tensor(
                nmr[:], ndx[:], 0.0, ndy[:], op0=Alu.max, op1=Alu.max
            )
            # dneg = inter - (area_t+EPS) - area_p = -denom ; rcp = -1/denom
            nc.vector.scalar_tensor_tensor(
                dneg[:], inter[:], ate, AREAP, op0=Alu.subtract, op1=Alu.subtract
            )
            nc.vector.reciprocal(rcp[:], dneg[:])
            # fin = (-inter)*rcp = iou
            nc.vector.scalar_tensor_tensor(
                fin[:], inter[:], -1.0, rcp[:], op0=Alu.mult, op1=Alu.mult
            )
            # where no overlap, take -mse from PSUM
            nc.vector.copy_predicated(fin[:], nmr[:].bitcast(i32), nl1[:])

            nc.vector.max(out=VALS[:, j, :], in_=fin[:])
            nc.vector.max_index(IDX[:, j, :], VALS[:, j, :], fin[:])

        nc.sync.dma_start(vals_d[:].rearrange("(j p) k -> p j k", p=128), VALS[:])
        nc.sync.dma_start(idx_d[:].rearrange("(j p) k -> p j k", p=128), IDX[:])

    nc.compile()
    return nc


def _prep_core_inputs(pred, tgt):
    """Host-side O(P+T) derived quantities. pred [P,4], tgt [T,4] float32."""
    shared = {}
    x0t = tgt[:, 0] - tgt[:, 2] / 2
    x1t = tgt[:, 0] + tgt[:, 2] / 2
    y0t = tgt[:, 1] - tgt[:, 3] / 2
    y1t = tgt[:, 1] + tgt[:, 3] / 2
    ate = tgt[:, 2] * tgt[:, 3] + np.float32(EPS)
    qt4 = np.sum(tgt * tgt, axis=-1) / 4
    ones_t = np.ones_like(qt4)
    shared["tscal"] = np.ascontiguousarray(
        np.stack([x0t, x1t, y0t, y1t, ate]).astype(np.float32)
    )
    shared["tgtT"] = np.ascontiguousarray(
        np.concatenate([tgt.T, ones_t[None, :], -qt4[None, :]]).astype(np.float32)
    )

    in_maps = []
    for c in range(N_CORES):
        sh = pred[c * P_CORE : (c + 1) * P_CORE]
        x0p = np.maximum(sh[:, 0] - sh[:, 2] / 2, np.float32(0.0))
        x1p = np.minimum(sh[:, 0] + sh[:, 2] / 2, np.float32(1.0))
        y0p = np.maximum(sh[:, 1] - sh[:, 3] / 2, np.float32(0.0))
        y1p = np.minimum(sh[:, 1] + sh[:, 3] / 2, np.float32(1.0))
        areap = sh[:, 2] * sh[:, 3]
        qp = np.sum(sh * sh, axis=-1) / 4
        ones_p = np.ones_like(qp)
        in_maps.append(
            {
                "pshard": np.ascontiguousarray(
                    np.stack([x0p, x1p, y0p, y1p, areap]).astype(np.float32)
                ).reshape(1, 5 * P_CORE),
                "hpredT": np.ascontiguousarray(
                    np.concatenate(
                        [0.5 * sh.T, -qp[None, :], ones_p[None, :]]
                    ).astype(np.float32)
                ),
                **shared,
            }
        )
    return in_maps


def _row_loss(pred, trow):
    """Exact device-form loss of one target row vs all preds (numpy f32)."""
    x0p = np.maximum(pred[:, 0] - pred[:, 2] / 2, np.float32(0.0))
    x1p = np.minimum(pred[:, 0] + pred[:, 2] / 2, np.float32(1.0))
    y0p = np.maximum(pred[:, 1] - pred[:, 3] / 2, np.float32(0.0))
    y1p = np.minimum(pred[:, 1] + pred[:, 3] / 2, np.float32(1.0))
    areap = pred[:, 2] * pred[:, 3]
    x0t = trow[0] - trow[2] / 2
    x1t = trow[0] + trow[2] / 2
    y0t = trow[1] - trow[3] / 2
    y1t = trow[1] + trow[3] / 2
    ndx = np.maximum(x0p, x0t) - np.minimum(x1p, x1t)
    ndy = np.maximum(y0p, y0t) - np.minimum(y1p, y1t)
    inter = ndx * ndy
    nov = np.maximum(ndx, ndy) > 0
    dneg = (inter - (trow[2] * trow[3] + np.float32(EPS))) - areap
    with np.errstate(divide="ignore", invalid="ignore"):
        iou = (-inter) * np.reciprocal(dneg)
    cross = pred @ (0.5 * trow).astype(np.float32)
    nmse = (cross - np.sum(trow * trow) / 4) - np.sum(pred * pred, axis=-1) / 4
    v = np.where(nov, nmse, iou)  # device value; loss = 1 - v
    return (np.float32(1.0) - v).astype(np.float32)


def _host_greedy(vals, idxs, pred, tgt):
    """vals [T, 8, 8] f32 (device v, descending), idxs [T, 8, 8] local indices."""
    loss = (1.0 - vals.reshape(T, N_CORES * TOPK).astype(np.float64))
    gidx = (
        idxs.astype(np.int64)
        + (np.arange(N_CORES)[None, :, None] * P_CORE)
    ).reshape(T, N_CORES * TOPK)

    taken = np.zeros(P_TOTAL, dtype=bool)
    total = 0.0
    for t in range(T):
        lt, gt = loss[t], gidx[t]
        order = np.lexsort((gt, lt))
        chosen = -1
        depth = 0
        for d in order:
            if not taken[gt[d]]:
                chosen = d
                break
            depth += 1
        safe = chosen >= 0
        if safe and depth >= TOPK:
            # a fully-taken shard whose worst listed candidate is better than
            # our choice could hide the true argmin
            closs = lt[chosen]
            for s in range(N_CORES):
                blk = slice(s * TOPK, (s + 1) * TOPK)
                if lt[s * TOPK + TOPK - 1] < closs and taken[gt[blk]].all():
                    safe = False
                    break
        if safe:
            k = gt[chosen]
            closs = lt[chosen]
        else:
            row = _row_loss(pred, tgt[t]).astype(np.float64)
            row[taken] = np.inf
            k = int(np.argmin(row))
            closs = row[k]
        taken[k] = True
        total += closs
    return np.float32(total / T)


def kernel(pred_bboxes, target_bboxes):
    from concourse.bass_utils import run_bass_kernel_spmd

    pred = np.asarray(pred_bboxes, dtype=np.float32)[0]
    tgt = np.asarray(target_bboxes, dtype=np.float32)[0]

    if "nc" not in _CACHE:
        _CACHE["nc"] = _build_nc()
    nc = _CACHE["nc"]

    in_maps = _prep_core_inputs(pred, tgt)
    res = run_bass_kernel_spmd(nc, in_maps, list(range(N_CORES)))
    results = res.results
    vals = np.stack([results[c]["cand_vals"] for c in range(N_CORES)], axis=1)
    idxs = np.stack([results[c]["cand_idx"] for c in range(N_CORES)], axis=1)
    return _host_greedy(vals, idxs, pred, tgt)


# revision 8
# speedup vs baseline: 1.0175x; 1.0175x over previous
"""Trainium2 Bass kernel: greedy bbox-matching loss (nn_BboxLoss).

Full computation: L[t,p] = pairwise bbox loss (IoU / MSE mix), then greedy
per-target argmin over still-available preds, mean of selected losses.

Strategy (8 NeuronCores, preds sharded 8 x 1024):
  device: per core, compute negated-loss tiles [128 targets x 1024 preds]
          entirely in SBUF, then top-8 per target via max/max_index.
          - PE matmul (K=6) produces -mse directly in PSUM (coords cross-term
            plus folded -sq_p/4 and -sq_t/4 rows).
          - ACT computes the min(corner) terms via warm-table Relu chains.
          - DVE does the remaining elementwise passes + top-8 extraction.
  host:   merge the 8x8=64 candidates per target, run the (inherently
          sequential, O(T*64)) greedy walk with an exactness safety check;
          rare unsafe rows fall back to a full-row recompute.

Device returns v = iou (overlap) or -mse (no overlap); true loss = 1 - v.
"""
import numpy as np
from contextlib import ExitStack

P_TOTAL = 8192
T = 2048
N_CORES = 8
NP_SHARD = 4                  # pred shards
NT_SHARD = 2                  # target shards
P_CORE = P_TOTAL // NP_SHARD  # 2048 preds per core
T_CORE = T // NT_SHARD        # 1024 targets per core
NJ = T_CORE // 128            # 8 row tiles of 128 targets
EPS = 1e-7
TOPK = 8

_CACHE = {}


def _build_nc():
    import concourse.bacc as bacc
    import concourse.mybir as mybir
    from concourse.tile import TileContext

    f32 = mybir.dt.float32
    u32 = mybir.dt.uint32
    i32 = mybir.dt.int32
    Alu = mybir.AluOpType
    Act = mybir.ActivationFunctionType

    nc = bacc.Bacc()
    ps_d = nc.dram_tensor("pshard", [1, 5 * P_CORE], f32, kind="ExternalInput")
    hp_d = nc.dram_tensor("hpredT", [6, P_CORE], f32, kind="ExternalInput")
    tsc_d = nc.dram_tensor("tscal", [5, T_CORE], f32, kind="ExternalInput")
    tt_d = nc.dram_tensor("tgtT", [6, T_CORE], f32, kind="ExternalInput")
    vals_d = nc.dram_tensor("cand_vals", [T_CORE, TOPK], f32, kind="ExternalOutput")
    idx_d = nc.dram_tensor("cand_idx", [T_CORE, TOPK], u32, kind="ExternalOutput")

    with TileContext(nc) as tc, ExitStack() as ctx:
        const = ctx.enter_context(tc.tile_pool(name="const", bufs=1))
        work = ctx.enter_context(tc.tile_pool(name="work", bufs=2))
        psum = ctx.enter_context(
            tc.tile_pool(name="psum", bufs=2, space="PSUM")
        )

        HP = const.tile([6, P_CORE], f32)
        TSC = const.tile([128, 5, NJ], f32)
        TT6 = const.tile([6, T_CORE], f32)
        PLANES = const.tile([128, 5, P_CORE], f32)
        VALS = const.tile([128, NJ, TOPK], f32)
        IDX = const.tile([128, NJ, TOPK], u32)

        nc.sync.dma_start(HP[:], hp_d[:])
        nc.sync.dma_start(TSC[:], tsc_d[:].rearrange("q (j p) -> p q j", p=128))
        nc.sync.dma_start(TT6[:], tt_d[:])

        # load the five per-pred rows into partition 0 of PLANES, then
        # replicate across all 128 partitions in-place, one plane at a time,
        # ordered by first use so compute can start early
        PLF = PLANES[:].rearrange("p q n -> p (q n)")
        nc.sync.dma_start(PLF[0:1, :], ps_d[:])
        for q in (1, 3, 0, 2, 4):   # X1P, Y1P, X0P, Y0P, AREAP
            nc.gpsimd.partition_broadcast(
                PLANES[:, q, :], PLANES[0:1, q, :]
            )

        X0P = PLANES[:, 0, :]
        X1P = PLANES[:, 1, :]
        Y0P = PLANES[:, 2, :]
        Y1P = PLANES[:, 3, :]
        AREAP = PLANES[:, 4, :]

        for j in range(NJ):
            x0t = TSC[:, 0, j : j + 1]
            x1t = TSC[:, 1, j : j + 1]
            y0t = TSC[:, 2, j : j + 1]
            y1t = TSC[:, 3, j : j + 1]
            ate = TSC[:, 4, j : j + 1]   # area_t + EPS

            # nl1 = cross/2 - sq_t/4 - sq_p/4 = -mse, straight out of PE
            nl1 = psum.tile([128, P_CORE], f32, tag="nl1")
            lhsT = TT6[:, j * 128 : (j + 1) * 128]
            for h in range(P_CORE // 512):
                nc.tensor.matmul(
                    nl1[:, h * 512 : (h + 1) * 512],
                    lhsT,
                    HP[:, h * 512 : (h + 1) * 512],
                    start=True,
                    stop=True,
                )

            ox1 = work.tile([128, P_CORE], f32, tag="ox1")
            oy1 = work.tile([128, P_CORE], f32, tag="oy1")
            ndx = work.tile([128, P_CORE], f32, tag="ndx")
            ndy = work.tile([128, P_CORE], f32, tag="ndy")
            inter = work.tile([128, P_CORE], f32, tag="inter")
            nmr = work.tile([128, P_CORE], f32, tag="nmr")
            dneg2 = work.tile([128, P_CORE], f32, tag="dneg2x")
            rcp = work.tile([128, P_CORE], f32, tag="rcp")
            fin = work.tile([128, P_CORE], f32, tag="fin")

            # ox1 = min(x1p, x1t) = relu(x1t - relu(x1t - x1p)); warm Relu table
            nc.scalar.activation(ox1[:], X1P, Act.Relu, bias=x1t, scale=-1.0)
            nc.scalar.activation(ox1[:], ox1[:], Act.Relu, bias=x1t, scale=-1.0)
            nc.scalar.activation(oy1[:], Y1P, Act.Relu, bias=y1t, scale=-1.0)
            nc.scalar.activation(oy1[:], oy1[:], Act.Relu, bias=y1t, scale=-1.0)

            # ndx = max(x0p, x0t) - ox1  (= -dx)
            nc.vector.scalar_tensor_tensor(
                ndx[:], X0P, x0t, ox1[:], op0=Alu.max, op1=Alu.subtract
            )
            nc.vector.scalar_tensor_tensor(
                ndy[:], Y0P, y0t, oy1[:], op0=Alu.max, op1=Alu.subtract
            )
            # inter = ndx*ndy (= dx*dy)
            nc.vector.tensor_tensor(inter[:], ndx[:], ndy[:], op=Alu.mult)
            # nmr = relu(max(ndx, ndy)): bit pattern nonzero <=> no overlap
            nc.vector.scalar_tensor_tensor(
                nmr[:], ndx[:], 0.0, ndy[:], op0=Alu.max, op1=Alu.max
            )
            # dneg = inter - (area_t+EPS) - area_p = -denom ; rcp = -1/denom
            nc.vector.scalar_tensor_tensor(
                dneg2[:], inter[:], ate, AREAP, op0=Alu.subtract, op1=Alu.subtract
            )
            nc.vector.reciprocal(rcp[:], dneg2[:])
            # fin = (-inter)*rcp = iou
            nc.vector.scalar_tensor_tensor(
                fin[:], inter[:], -1.0, rcp[:], op0=Alu.mult, op1=Alu.mult
            )
            # where no overlap, take -mse from PSUM
            nc.vector.copy_predicated(fin[:], nmr[:].bitcast(i32), nl1[:])

            nc.vector.max(out=VALS[:, j, :], in_=fin[:])
            nc.vector.max_index(IDX[:, j, :], VALS[:, j, :], fin[:])

        nc.sync.dma_start(vals_d[:].rearrange("(j p) k -> p j k", p=128), VALS[:])
        nc.sync.dma_start(idx_d[:].rearrange("(j p) k -> p j k", p=128), IDX[:])

    nc.compile()
    return nc


def _prep_core_inputs(pred, tgt):
    """Host-side O(P+T) derived quantities. pred [P,4], tgt [T,4] float32."""
    shared = {}
    x0t = tgt[:, 0] - tgt[:, 2] / 2
    x1t = tgt[:, 0] + tgt[:, 2] / 2
    y0t = tgt[:, 1] - tgt[:, 3] / 2
    y1t = tgt[:, 1] + tgt[:, 3] / 2
    ate = tgt[:, 2] * tgt[:, 3] + np.float32(EPS)
    qt4 = np.sum(tgt * tgt, axis=-1) / 4
    ones_t = np.ones_like(qt4)
    shared["tscal"] = np.ascontiguousarray(
        np.stack([x0t, x1t, y0t, y1t, ate]).astype(np.float32)
    )
    shared["tgtT"] = np.ascontiguousarray(
        np.concatenate([tgt.T, ones_t[None, :], -qt4[None, :]]).astype(np.float32)
    )

    in_maps = []
    for c in range(N_CORES):
        px = c % NP_SHARD
        sh = pred[px * P_CORE : (px + 1) * P_CORE]
        x0p = np.maximum(sh[:, 0] - sh[:, 2] / 2, np.float32(0.0))
        x1p = np.minimum(sh[:, 0] + sh[:, 2] / 2, np.float32(1.0))
        y0p = np.maximum(sh[:, 1] - sh[:, 3] / 2, np.float32(0.0))
        y1p = np.minimum(sh[:, 1] + sh[:, 3] / 2, np.float32(1.0))
        areap = sh[:, 2] * sh[:, 3]
        qp = np.sum(sh * sh, axis=-1) / 4
        ones_p = np.ones_like(qp)
        ty = c // NP_SHARD
        tsl = slice(ty * T_CORE, (ty + 1) * T_CORE)
        in_maps.append(
            {
                "pshard": np.ascontiguousarray(
                    np.stack([x0p, x1p, y0p, y1p, areap]).astype(np.float32)
                ).reshape(1, 5 * P_CORE),
                "hpredT": np.ascontiguousarray(
                    np.concatenate(
                        [0.5 * sh.T, -qp[None, :], ones_p[None, :]]
                    ).astype(np.float32)
                ),
                "tscal": np.ascontiguousarray(shared["tscal"][:, tsl]),
                "tgtT": np.ascontiguousarray(shared["tgtT"][:, tsl]),
            }
        )
    return in_maps


def _row_loss(pred, trow):
    """Exact device-form loss of one target row vs all preds (numpy f32)."""
    x0p = np.maximum(pred[:, 0] - pred[:, 2] / 2, np.float32(0.0))
    x1p = np.minimum(pred[:, 0] + pred[:, 2] / 2, np.float32(1.0))
    y0p = np.maximum(pred[:, 1] - pred[:, 3] / 2, np.float32(0.0))
    y1p = np.minimum(pred[:, 1] + pred[:, 3] / 2, np.float32(1.0))
    areap = pred[:, 2] * pred[:, 3]
    x0t = trow[0] - trow[2] / 2
    x1t = trow[0] + trow[2] / 2
    y0t = trow[1] - trow[3] / 2
    y1t = trow[1] + trow[3] / 2
    ndx = np.maximum(x0p, x0t) - np.minimum(x1p, x1t)
    ndy = np.maximum(y0p, y0t) - np.minimum(y1p, y1t)
    inter = ndx * ndy
    nov = np.maximum(ndx, ndy) > 0
    dneg = (inter - (trow[2] * trow[3] + np.float32(EPS))) - areap
    with np.errstate(divide="ignore", invalid="ignore"):
        iou = (-inter) * np.reciprocal(dneg)
    cross = pred @ (0.5 * trow).astype(np.float32)
    nmse = (cross - np.sum(trow * trow) / 4) - np.sum(pred * pred, axis=-1) / 4
    v = np.where(nov, nmse, iou)  # device value; loss = 1 - v
    return (np.float32(1.0) - v).astype(np.float32)


def _host_greedy(vals, idxs, pred, tgt):
    """vals/idxs [T, NP_SHARD, TOPK]: per-target candidates from each pred shard."""
    NSH = NP_SHARD
    loss = (1.0 - vals.reshape(T, NSH * TOPK).astype(np.float64))
    gidx = (
        idxs.astype(np.int64)
        + (np.arange(NSH)[None, :, None] * P_CORE)
    ).reshape(T, NSH * TOPK)

    taken = np.zeros(P_TOTAL, dtype=bool)
    total = 0.0
    for t in range(T):
        lt, gt = loss[t], gidx[t]
        order = np.lexsort((gt, lt))
        chosen = -1
        depth = 0
        for d in order:
            if not taken[gt[d]]:
                chosen = d
                break
            depth += 1
        safe = chosen >= 0
        if safe and depth >= TOPK:
            # a fully-taken shard whose worst listed candidate is better than
            # our choice could hide the true argmin
            closs = lt[chosen]
            for s in range(NSH):
                blk = slice(s * TOPK, (s + 1) * TOPK)
                if lt[s * TOPK + TOPK - 1] < closs and taken[gt[blk]].all():
                    safe = False
                    break
        if safe:
            k = gt[chosen]
            closs = lt[chosen]
        else:
            row = _row_loss(pred, tgt[t]).astype(np.float64)
            row[taken] = np.inf
            k = int(np.argmin(row))
            closs = row[k]
        taken[k] = True
        total += closs
    return np.float32(total / T)


def kernel(pred_bboxes, target_bboxes):
    from concourse.bass_utils import run_bass_kernel_spmd

    pred = np.asarray(pred_bboxes, dtype=np.float32)[0]
    tgt = np.asarray(target_bboxes, dtype=np.float32)[0]

    if "nc" not in _CACHE:
        _CACHE["nc"] = _build_nc()
    nc = _CACHE["nc"]

    in_maps = _prep_core_inputs(pred, tgt)
    res = run_bass_kernel_spmd(nc, in_maps, list(range(N_CORES)))
    results = res.results
    # core c covers targets [ (c//NP) * T_CORE : ... ], pred shard c % NP
    vals = np.empty((T, NP_SHARD, TOPK), np.float32)
    idxs = np.empty((T, NP_SHARD, TOPK), np.uint32)
    for c in range(N_CORES):
        px, ty = c % NP_SHARD, c // NP_SHARD
        vals[ty * T_CORE : (ty + 1) * T_CORE, px] = results[c]["cand_vals"]
        idxs[ty * T_CORE : (ty + 1) * T_CORE, px] = results[c]["cand_idx"]
    return _host_greedy(vals, idxs, pred, tgt)


# revision 9
# speedup vs baseline: 1.0241x; 1.0065x over previous
"""Trainium2 Bass kernel: greedy bbox-matching loss (nn_BboxLoss).

Full computation: L[t,p] = pairwise bbox loss (IoU / MSE mix), then greedy
per-target argmin over still-available preds, mean of selected losses.

Strategy (8 NeuronCores, preds sharded 8 x 1024):
  device: per core, compute negated-loss tiles [128 targets x 1024 preds]
          entirely in SBUF, then top-8 per target via max/max_index.
          - PE matmul (K=6) produces -mse directly in PSUM (coords cross-term
            plus folded -sq_p/4 and -sq_t/4 rows).
          - ACT computes the min(corner) terms via warm-table Relu chains.
          - DVE does the remaining elementwise passes + top-8 extraction.
  host:   merge the 8x8=64 candidates per target, run the (inherently
          sequential, O(T*64)) greedy walk with an exactness safety check;
          rare unsafe rows fall back to a full-row recompute.

Device returns v = iou (overlap) or -mse (no overlap); true loss = 1 - v.
"""
import numpy as np
from contextlib import ExitStack

P_TOTAL = 8192
T = 2048
N_CORES = 8
NP_SHARD = 4                  # pred shards
NT_SHARD = 2                  # target shards
P_CORE = P_TOTAL // NP_SHARD  # 2048 preds per core
T_CORE = T // NT_SHARD        # 1024 targets per core
NJ = T_CORE // 128            # 8 row tiles of 128 targets
EPS = 1e-7
TOPK = 8

_CACHE = {}


def _build_nc():
    import concourse.bacc as bacc
    import concourse.mybir as mybir
    from concourse.tile import TileContext

    f32 = mybir.dt.float32
    u32 = mybir.dt.uint32
    i32 = mybir.dt.int32
    Alu = mybir.AluOpType
    Act = mybir.ActivationFunctionType

    nc = bacc.Bacc()
    ps_d = nc.dram_tensor("pshard", [1, 5 * P_CORE], f32, kind="ExternalInput")
    hp_d = nc.dram_tensor("hpredT", [6, P_CORE], f32, kind="ExternalInput")
    tsc_d = nc.dram_tensor("tscal", [128, 5 * NJ], f32, kind="ExternalInput")
    tt_d = nc.dram_tensor("tgtT", [6, T_CORE], f32, kind="ExternalInput")
    vals_d = nc.dram_tensor("cand_vals", [128, NJ * TOPK], f32, kind="ExternalOutput")
    idx_d = nc.dram_tensor("cand_idx", [128, NJ * TOPK], u32, kind="ExternalOutput")

    with TileContext(nc) as tc, ExitStack() as ctx:
        const = ctx.enter_context(tc.tile_pool(name="const", bufs=1))
        work = ctx.enter_context(tc.tile_pool(name="work", bufs=2))
        psum = ctx.enter_context(
            tc.tile_pool(name="psum", bufs=2, space="PSUM")
        )

        HP = const.tile([6, P_CORE], f32)
        TSC = const.tile([128, 5, NJ], f32)
        TT6 = const.tile([6, T_CORE], f32)
        PLANES = const.tile([128, 5, P_CORE], f32)
        VALS = const.tile([128, NJ, TOPK], f32)
        IDX = const.tile([128, NJ, TOPK], u32)

        nc.sync.dma_start(HP[:], hp_d[:])
        nc.sync.dma_start(TSC[:].rearrange("p q j -> p (q j)"), tsc_d[:])
        nc.sync.dma_start(TT6[:], tt_d[:])

        # load the five per-pred rows into partition 0 of PLANES, then
        # replicate across all 128 partitions in-place, one plane at a time,
        # ordered by first use so compute can start early
        PLF = PLANES[:].rearrange("p q n -> p (q n)")
        nc.sync.dma_start(PLF[0:1, :], ps_d[:])
        for q in (1, 3, 0, 2, 4):   # X1P, Y1P, X0P, Y0P, AREAP
            nc.gpsimd.partition_broadcast(
                PLANES[:, q, :], PLANES[0:1, q, :]
            )

        X0P = PLANES[:, 0, :]
        X1P = PLANES[:, 1, :]
        Y0P = PLANES[:, 2, :]
        Y1P = PLANES[:, 3, :]
        AREAP = PLANES[:, 4, :]

        for j in range(NJ):
            x0t = TSC[:, 0, j : j + 1]
            x1t = TSC[:, 1, j : j + 1]
            y0t = TSC[:, 2, j : j + 1]
            y1t = TSC[:, 3, j : j + 1]
            ate = TSC[:, 4, j : j + 1]   # area_t + EPS

            # nl1 = cross/2 - sq_t/4 - sq_p/4 = -mse, straight out of PE
            nl1 = psum.tile([128, P_CORE], f32, tag="nl1")
            lhsT = TT6[:, j * 128 : (j + 1) * 128]
            for h in range(P_CORE // 512):
                nc.tensor.matmul(
                    nl1[:, h * 512 : (h + 1) * 512],
                    lhsT,
                    HP[:, h * 512 : (h + 1) * 512],
                    start=True,
                    stop=True,
                )

            ox1 = work.tile([128, P_CORE], f32, tag="ox1")
            oy1 = work.tile([128, P_CORE], f32, tag="oy1")
            ndx = work.tile([128, P_CORE], f32, tag="ndx")
            ndy = work.tile([128, P_CORE], f32, tag="ndy")
            inter = work.tile([128, P_CORE], f32, tag="inter")
            nmr = work.tile([128, P_CORE], f32, tag="nmr")
            dneg2 = work.tile([128, P_CORE], f32, tag="dneg2x")
            rcp = work.tile([128, P_CORE], f32, tag="rcp")
            fin = work.tile([128, P_CORE], f32, tag="fin")

            # ox1 = min(x1p, x1t) = relu(x1t - relu(x1t - x1p)); warm Relu table
            nc.scalar.activation(ox1[:], X1P, Act.Relu, bias=x1t, scale=-1.0)
            nc.scalar.activation(ox1[:], ox1[:], Act.Relu, bias=x1t, scale=-1.0)
            nc.scalar.activation(oy1[:], Y1P, Act.Relu, bias=y1t, scale=-1.0)
            nc.scalar.activation(oy1[:], oy1[:], Act.Relu, bias=y1t, scale=-1.0)

            # ndx = max(x0p, x0t) - ox1  (= -dx)
            nc.vector.scalar_tensor_tensor(
                ndx[:], X0P, x0t, ox1[:], op0=Alu.max, op1=Alu.subtract
            )
            nc.vector.scalar_tensor_tensor(
                ndy[:], Y0P, y0t, oy1[:], op0=Alu.max, op1=Alu.subtract
            )
            # inter = ndx*ndy (= dx*dy)
            nc.vector.tensor_tensor(inter[:], ndx[:], ndy[:], op=Alu.mult)
            # nmr = relu(max(ndx, ndy)): bit pattern nonzero <=> no overlap
            nc.vector.scalar_tensor_tensor(
                nmr[:], ndx[:], 0.0, ndy[:], op0=Alu.max, op1=Alu.max
            )
            # dneg = inter - (area_t+EPS) - area_p = -denom ; rcp = -1/denom
            nc.vector.scalar_tensor_tensor(
                dneg2[:], inter[:], ate, AREAP, op0=Alu.subtract, op1=Alu.subtract
            )
            nc.vector.reciprocal(rcp[:], dneg2[:])
            # fin = (-inter)*rcp = iou
            nc.vector.scalar_tensor_tensor(
                fin[:], inter[:], -1.0, rcp[:], op0=Alu.mult, op1=Alu.mult
            )
            # where no overlap, take -mse from PSUM
            nc.vector.copy_predicated(fin[:], nmr[:].bitcast(i32), nl1[:])

            nc.vector.max(out=VALS[:, j, :], in_=fin[:])
            nc.vector.max_index(IDX[:, j, :], VALS[:, j, :], fin[:])

        nc.sync.dma_start(vals_d[:], VALS[:].rearrange("p j k -> p (j k)"))
        nc.sync.dma_start(idx_d[:], IDX[:].rearrange("p j k -> p (j k)"))

    nc.compile()
    return nc


def _prep_core_inputs(pred, tgt):
    """Host-side O(P+T) derived quantities. pred [P,4], tgt [T,4] float32."""
    shared = {}
    x0t = tgt[:, 0] - tgt[:, 2] / 2
    x1t = tgt[:, 0] + tgt[:, 2] / 2
    y0t = tgt[:, 1] - tgt[:, 3] / 2
    y1t = tgt[:, 1] + tgt[:, 3] / 2
    ate = tgt[:, 2] * tgt[:, 3] + np.float32(EPS)
    qt4 = np.sum(tgt * tgt, axis=-1) / 4
    ones_t = np.ones_like(qt4)
    tscal = np.stack([x0t, x1t, y0t, y1t, ate]).astype(np.float32)  # [5, T]
    shared["tscal"] = tscal
    shared["tgtT"] = np.ascontiguousarray(
        np.concatenate([tgt.T, ones_t[None, :], -qt4[None, :]]).astype(np.float32)
    )

    in_maps = []
    for c in range(N_CORES):
        px = c % NP_SHARD
        sh = pred[px * P_CORE : (px + 1) * P_CORE]
        x0p = np.maximum(sh[:, 0] - sh[:, 2] / 2, np.float32(0.0))
        x1p = np.minimum(sh[:, 0] + sh[:, 2] / 2, np.float32(1.0))
        y0p = np.maximum(sh[:, 1] - sh[:, 3] / 2, np.float32(0.0))
        y1p = np.minimum(sh[:, 1] + sh[:, 3] / 2, np.float32(1.0))
        areap = sh[:, 2] * sh[:, 3]
        qp = np.sum(sh * sh, axis=-1) / 4
        ones_p = np.ones_like(qp)
        ty = c // NP_SHARD
        tsl = slice(ty * T_CORE, (ty + 1) * T_CORE)
        in_maps.append(
            {
                "pshard": np.ascontiguousarray(
                    np.stack([x0p, x1p, y0p, y1p, areap]).astype(np.float32)
                ).reshape(1, 5 * P_CORE),
                "hpredT": np.ascontiguousarray(
                    np.concatenate(
                        [0.5 * sh.T, -qp[None, :], ones_p[None, :]]
                    ).astype(np.float32)
                ),
                "tscal": np.ascontiguousarray(
                    shared["tscal"][:, tsl].reshape(5, -1, 128).transpose(2, 0, 1)
                    .reshape(128, -1)
                ),
                "tgtT": np.ascontiguousarray(shared["tgtT"][:, tsl]),
            }
        )
    return in_maps


def _row_loss(pred, trow):
    """Exact device-form loss of one target row vs all preds (numpy f32)."""
    x0p = np.maximum(pred[:, 0] - pred[:, 2] / 2, np.float32(0.0))
    x1p = np.minimum(pred[:, 0] + pred[:, 2] / 2, np.float32(1.0))
    y0p = np.maximum(pred[:, 1] - pred[:, 3] / 2, np.float32(0.0))
    y1p = np.minimum(pred[:, 1] + pred[:, 3] / 2, np.float32(1.0))
    areap = pred[:, 2] * pred[:, 3]
    x0t = trow[0] - trow[2] / 2
    x1t = trow[0] + trow[2] / 2
    y0t = trow[1] - trow[3] / 2
    y1t = trow[1] + trow[3] / 2
    ndx = np.maximum(x0p, x0t) - np.minimum(x1p, x1t)
    ndy = np.maximum(y0p, y0t) - np.minimum(y1p, y1t)
    inter = ndx * ndy
    nov = np.maximum(ndx, ndy) > 0
    dneg = (inter - (trow[2] * trow[3] + np.float32(EPS))) - areap
    with np.errstate(divide="ignore", invalid="ignore"):
        iou = (-inter) * np.reciprocal(dneg)
    cross = pred @ (0.5 * trow).astype(np.float32)
    nmse = (cross - np.sum(trow * trow) / 4) - np.sum(pred * pred, axis=-1) / 4
    v = np.where(nov, nmse, iou)  # device value; loss = 1 - v
    return (np.float32(1.0) - v).astype(np.float32)


def _host_greedy(vals, idxs, pred, tgt):
    """vals/idxs [T, NP_SHARD, TOPK]: per-target candidates from each pred shard."""
    NSH = NP_SHARD
    loss = (1.0 - vals.reshape(T, NSH * TOPK).astype(np.float64))
    gidx = (
        idxs.astype(np.int64)
        + (np.arange(NSH)[None, :, None] * P_CORE)
    ).reshape(T, NSH * TOPK)

    taken = np.zeros(P_TOTAL, dtype=bool)
    total = 0.0
    for t in range(T):
        lt, gt = loss[t], gidx[t]
        order = np.lexsort((gt, lt))
        chosen = -1
        depth = 0
        for d in order:
            if not taken[gt[d]]:
                chosen = d
                break
            depth += 1
        safe = chosen >= 0
        if safe and depth >= TOPK:
            # a fully-taken shard whose worst listed candidate is better than
            # our choice could hide the true argmin
            closs = lt[chosen]
            for s in range(NSH):
                blk = slice(s * TOPK, (s + 1) * TOPK)
                if lt[s * TOPK + TOPK - 1] < closs and taken[gt[blk]].all():
                    safe = False
                    break
        if safe:
            k = gt[chosen]
            closs = lt[chosen]
        else:
            row = _row_loss(pred, tgt[t]).astype(np.float64)
            row[taken] = np.inf
            k = int(np.argmin(row))
            closs = row[k]
        taken[k] = True
        total += closs
    return np.float32(total / T)


def kernel(pred_bboxes, target_bboxes):
    from concourse.bass_utils import run_bass_kernel_spmd

    pred = np.asarray(pred_bboxes, dtype=np.float32)[0]
    tgt = np.asarray(target_bboxes, dtype=np.float32)[0]

    if "nc" not in _CACHE:
        _CACHE["nc"] = _build_nc()
    nc = _CACHE["nc"]

    in_maps = _prep_core_inputs(pred, tgt)
    res = run_bass_kernel_spmd(nc, in_maps, list(range(N_CORES)))
    results = res.results
    # core c covers targets [ (c//NP) * T_CORE : ... ], pred shard c % NP
    vals = np.empty((T, NP_SHARD, TOPK), np.float32)
    idxs = np.empty((T, NP_SHARD, TOPK), np.uint32)
    def _deint(a):
        # [128, NJ*TOPK] -> [T_CORE, TOPK]; target t = j*128 + p
        return (
            a.reshape(128, -1, TOPK).transpose(1, 0, 2).reshape(T_CORE, TOPK)
        )
    for c in range(N_CORES):
        px, ty = c % NP_SHARD, c // NP_SHARD
        vals[ty * T_CORE : (ty + 1) * T_CORE, px] = _deint(results[c]["cand_vals"])
        idxs[ty * T_CORE : (ty + 1) * T_CORE, px] = _deint(results[c]["cand_idx"])
    return _host_greedy(vals, idxs, pred, tgt)


# revision 10
# speedup vs baseline: 1.0246x; 1.0005x over previous
"""Trainium2 Bass kernel: greedy bbox-matching loss (nn_BboxLoss).

Full computation: L[t,p] = pairwise bbox loss (IoU / MSE mix), then greedy
per-target argmin over still-available preds, mean of selected losses.

Strategy (8 NeuronCores, preds sharded 8 x 1024):
  device: per core, compute negated-loss tiles [128 targets x 1024 preds]
          entirely in SBUF, then top-8 per target via max/max_index.
          - PE matmul (K=6) produces -mse directly in PSUM (coords cross-term
            plus folded -sq_p/4 and -sq_t/4 rows).
          - ACT computes the min(corner) terms via warm-table Relu chains.
          - DVE does the remaining elementwise passes + top-8 extraction.
  host:   merge the 8x8=64 candidates per target, run the (inherently
          sequential, O(T*64)) greedy walk with an exactness safety check;
          rare unsafe rows fall back to a full-row recompute.

Device returns v = iou (overlap) or -mse (no overlap); true loss = 1 - v.
"""
import numpy as np
from contextlib import ExitStack

P_TOTAL = 8192
T = 2048
N_CORES = 8
NP_SHARD = 4                  # pred shards
NT_SHARD = 2                  # target shards
P_CORE = P_TOTAL // NP_SHARD  # 2048 preds per core
T_CORE = T // NT_SHARD        # 1024 targets per core
NJ = T_CORE // 128            # 8 row tiles of 128 targets
EPS = 1e-7
TOPK = 8

_CACHE = {}


def _build_nc():
    import concourse.bacc as bacc
    import concourse.mybir as mybir
    from concourse.tile import TileContext

    f32 = mybir.dt.float32
    u32 = mybir.dt.uint32
    i32 = mybir.dt.int32
    Alu = mybir.AluOpType
    Act = mybir.ActivationFunctionType

    nc = bacc.Bacc()
    ps_d = nc.dram_tensor("pshard", [1, 5 * P_CORE], f32, kind="ExternalInput")
    hp_d = nc.dram_tensor("hpredT", [6, P_CORE], f32, kind="ExternalInput")
    tsc_d = nc.dram_tensor("tscal", [128, 5 * NJ], f32, kind="ExternalInput")
    tt_d = nc.dram_tensor("tgtT", [6, T_CORE], f32, kind="ExternalInput")
    vals_d = nc.dram_tensor("cand_vals", [128, NJ * TOPK], f32, kind="ExternalOutput")
    idx_d = nc.dram_tensor("cand_idx", [128, NJ * TOPK], u32, kind="ExternalOutput")

    with TileContext(nc) as tc, ExitStack() as ctx:
        const = ctx.enter_context(tc.tile_pool(name="const", bufs=1))
        work = ctx.enter_context(tc.tile_pool(name="work", bufs=2))
        psum = ctx.enter_context(
            tc.tile_pool(name="psum", bufs=2, space="PSUM")
        )

        HP = const.tile([6, P_CORE], f32)
        TSC = const.tile([128, 5, NJ], f32)
        TT6 = const.tile([6, T_CORE], f32)
        PLANES = const.tile([128, 5, P_CORE], f32)
        VALS = const.tile([128, NJ, TOPK], f32)
        IDX = const.tile([128, NJ, TOPK], u32)

        nc.sync.dma_start(HP[:], hp_d[:])
        nc.sync.dma_start(TSC[:].rearrange("p q j -> p (q j)"), tsc_d[:])
        nc.sync.dma_start(TT6[:], tt_d[:])

        # load the five per-pred rows into partition 0 of PLANES, then
        # replicate across all 128 partitions in-place, one plane at a time,
        # ordered by first use so compute can start early
        PLF = PLANES[:].rearrange("p q n -> p (q n)")
        nc.sync.dma_start(PLF[0:1, :], ps_d[:])
        for q in (1, 3, 0, 2, 4):   # X1P, Y1P, X0P, Y0P, AREAP
            nc.gpsimd.partition_broadcast(
                PLANES[:, q, :], PLANES[0:1, q, :]
            )

        X0P = PLANES[:, 0, :]
        X1P = PLANES[:, 1, :]
        Y0P = PLANES[:, 2, :]
        Y1P = PLANES[:, 3, :]
        AREAP = PLANES[:, 4, :]

        for j in range(NJ):
            x0t = TSC[:, 0, j : j + 1]
            x1t = TSC[:, 1, j : j + 1]
            y0t = TSC[:, 2, j : j + 1]
            y1t = TSC[:, 3, j : j + 1]
            ate = TSC[:, 4, j : j + 1]   # area_t + EPS

            # nl1 = cross/2 - sq_t/4 - sq_p/4 = -mse, straight out of PE
            nl1 = psum.tile([128, P_CORE], f32, tag="nl1")
            lhsT = TT6[:, j * 128 : (j + 1) * 128]
            for h in range(P_CORE // 512):
                nc.tensor.matmul(
                    nl1[:, h * 512 : (h + 1) * 512],
                    lhsT,
                    HP[:, h * 512 : (h + 1) * 512],
                    start=True,
                    stop=True,
                )

            ox1 = work.tile([128, P_CORE], f32, tag="ox1")
            oy1 = work.tile([128, P_CORE], f32, tag="oy1")
            ndx = work.tile([128, P_CORE], f32, tag="ndx")
            ndy = work.tile([128, P_CORE], f32, tag="ndy")
            inter = work.tile([128, P_CORE], f32, tag="inter")
            nmr = work.tile([128, P_CORE], f32, tag="nmr")
            dneg2 = work.tile([128, P_CORE], f32, tag="dneg2x")
            rcp = work.tile([128, P_CORE], f32, tag="rcp")
            fin = work.tile([128, P_CORE], f32, tag="fin")

            # ox1 = min(x1p, x1t) = relu(x1t - relu(x1t - x1p)); warm Relu table
            nc.scalar.activation(ox1[:], X1P, Act.Relu, bias=x1t, scale=-1.0)
            nc.scalar.activation(ox1[:], ox1[:], Act.Relu, bias=x1t, scale=-1.0)
            nc.scalar.activation(oy1[:], Y1P, Act.Relu, bias=y1t, scale=-1.0)
            nc.scalar.activation(oy1[:], oy1[:], Act.Relu, bias=y1t, scale=-1.0)

            # ndx = max(x0p, x0t) - ox1  (= -dx)
            nc.vector.scalar_tensor_tensor(
                ndx[:], X0P, x0t, ox1[:], op0=Alu.max, op1=Alu.subtract
            )
            nc.vector.scalar_tensor_tensor(
                ndy[:], Y0P, y0t, oy1[:], op0=Alu.max, op1=Alu.subtract
            )
            # nmr = relu(max(ndx, ndy)): bit pattern nonzero <=> no overlap
            nc.vector.scalar_tensor_tensor(
                nmr[:], ndx[:], 0.0, ndy[:], op0=Alu.max, op1=Alu.max
            )
            # inter = ndx*ndy (= dx*dy)
            nc.vector.tensor_tensor(inter[:], ndx[:], ndy[:], op=Alu.mult)
            # dneg = inter - (area_t+EPS) - area_p = -denom ; rcp = -1/denom
            nc.vector.scalar_tensor_tensor(
                dneg2[:], inter[:], ate, AREAP, op0=Alu.subtract, op1=Alu.subtract
            )
            nc.vector.reciprocal(rcp[:], dneg2[:])
            # fin = (-inter)*rcp = iou
            nc.vector.scalar_tensor_tensor(
                fin[:], inter[:], -1.0, rcp[:], op0=Alu.mult, op1=Alu.mult
            )
            # where no overlap, take -mse from PSUM
            nc.vector.copy_predicated(fin[:], nmr[:].bitcast(i32), nl1[:])

            nc.vector.max(out=VALS[:, j, :], in_=fin[:])
            nc.vector.max_index(IDX[:, j, :], VALS[:, j, :], fin[:])

        nc.sync.dma_start(vals_d[:], VALS[:].rearrange("p j k -> p (j k)"))
        nc.sync.dma_start(idx_d[:], IDX[:].rearrange("p j k -> p (j k)"))

    nc.compile()
    return nc


def _prep_core_inputs(pred, tgt):
    """Host-side O(P+T) derived quantities. pred [P,4], tgt [T,4] float32."""
    shared = {}
    x0t = tgt[:, 0] - tgt[:, 2] / 2
    x1t = tgt[:, 0] + tgt[:, 2] / 2
    y0t = tgt[:, 1] - tgt[:, 3] / 2
    y1t = tgt[:, 1] + tgt[:, 3] / 2
    ate = tgt[:, 2] * tgt[:, 3] + np.float32(EPS)
    qt4 = np.sum(tgt * tgt, axis=-1) / 4
    ones_t = np.ones_like(qt4)
    tscal = np.stack([x0t, x1t, y0t, y1t, ate]).astype(np.float32)  # [5, T]
    shared["tscal"] = tscal
    shared["tgtT"] = np.ascontiguousarray(
        np.concatenate([tgt.T, ones_t[None, :], -qt4[None, :]]).astype(np.float32)
    )

    in_maps = []
    for c in range(N_CORES):
        px = c % NP_SHARD
        sh = pred[px * P_CORE : (px + 1) * P_CORE]
        x0p = np.maximum(sh[:, 0] - sh[:, 2] / 2, np.float32(0.0))
        x1p = np.minimum(sh[:, 0] + sh[:, 2] / 2, np.float32(1.0))
        y0p = np.maximum(sh[:, 1] - sh[:, 3] / 2, np.float32(0.0))
        y1p = np.minimum(sh[:, 1] + sh[:, 3] / 2, np.float32(1.0))
        areap = sh[:, 2] * sh[:, 3]
        qp = np.sum(sh * sh, axis=-1) / 4
        ones_p = np.ones_like(qp)
        ty = c // NP_SHARD
        tsl = slice(ty * T_CORE, (ty + 1) * T_CORE)
        in_maps.append(
            {
                "pshard": np.ascontiguousarray(
                    np.stack([x0p, x1p, y0p, y1p, areap]).astype(np.float32)
                ).reshape(1, 5 * P_CORE),
                "hpredT": np.ascontiguousarray(
                    np.concatenate(
                        [0.5 * sh.T, -qp[None, :], ones_p[None, :]]
                    ).astype(np.float32)
                ),
                "tscal": np.ascontiguousarray(
                    shared["tscal"][:, tsl].reshape(5, -1, 128).transpose(2, 0, 1)
                    .reshape(128, -1)
                ),
                "tgtT": np.ascontiguousarray(shared["tgtT"][:, tsl]),
            }
        )
    return in_maps


def _row_loss(pred, trow):
    """Exact device-form loss of one target row vs all preds (numpy f32)."""
    x0p = np.maximum(pred[:, 0] - pred[:, 2] / 2, np.float32(0.0))
    x1p = np.minimum(pred[:, 0] + pred[:, 2] / 2, np.float32(1.0))
    y0p = np.maximum(pred[:, 1] - pred[:, 3] / 2, np.float32(0.0))
    y1p = np.minimum(pred[:, 1] + pred[:, 3] / 2, np.float32(1.0))
    areap = pred[:, 2] * pred[:, 3]
    x0t = trow[0] - trow[2] / 2
    x1t = trow[0] + trow[2] / 2
    y0t = trow[1] - trow[3] / 2
    y1t = trow[1] + trow[3] / 2
    ndx = np.maximum(x0p, x0t) - np.minimum(x1p, x1t)
    ndy = np.maximum(y0p, y0t) - np.minimum(y1p, y1t)
    inter = ndx * ndy
    nov = np.maximum(ndx, ndy) > 0
    dneg = (inter - (trow[2] * trow[3] + np.float32(EPS))) - areap
    with np.errstate(divide="ignore", invalid="ignore"):
        iou = (-inter) * np.reciprocal(dneg)
    cross = pred @ (0.5 * trow).astype(np.float32)
    nmse = (cross - np.sum(trow * trow) / 4) - np.sum(pred * pred, axis=-1) / 4
    v = np.where(nov, nmse, iou)  # device value; loss = 1 - v
    return (np.float32(1.0) - v).astype(np.float32)


def _host_greedy(vals, idxs, pred, tgt):
    """vals/idxs [T, NP_SHARD, TOPK]: per-target candidates from each pred shard."""
    NSH = NP_SHARD
    loss = (1.0 - vals.reshape(T, NSH * TOPK).astype(np.float64))
    gidx = (
        idxs.astype(np.int64)
        + (np.arange(NSH)[None, :, None] * P_CORE)
    ).reshape(T, NSH * TOPK)

    taken = np.zeros(P_TOTAL, dtype=bool)
    total = 0.0
    for t in range(T):
        lt, gt = loss[t], gidx[t]
        order = np.lexsort((gt, lt))
        chosen = -1
        depth = 0
        for d in order:
            if not taken[gt[d]]:
                chosen = d
                break
            depth += 1
        safe = chosen >= 0
        if safe and depth >= TOPK:
            # a fully-taken shard whose worst listed candidate is better than
            # our choice could hide the true argmin
            closs = lt[chosen]
            for s in range(NSH):
                blk = slice(s * TOPK, (s + 1) * TOPK)
                if lt[s * TOPK + TOPK - 1] < closs and taken[gt[blk]].all():
                    safe = False
                    break
        if safe:
            k = gt[chosen]
            closs = lt[chosen]
        else:
            row = _row_loss(pred, tgt[t]).astype(np.float64)
            row[taken] = np.inf
            k = int(np.argmin(row))
            closs = row[k]
        taken[k] = True
        total += closs
    return np.float32(total / T)


def kernel(pred_bboxes, target_bboxes):
    from concourse.bass_utils import run_bass_kernel_spmd

    pred = np.asarray(pred_bboxes, dtype=np.float32)[0]
    tgt = np.asarray(target_bboxes, dtype=np.float32)[0]

    if "nc" not in _CACHE:
        _CACHE["nc"] = _build_nc()
    nc = _CACHE["nc"]

    in_maps = _prep_core_inputs(pred, tgt)
    res = run_bass_kernel_spmd(nc, in_maps, list(range(N_CORES)))
    results = res.results
    # core c covers targets [ (c//NP) * T_CORE : ... ], pred shard c % NP
    vals = np.empty((T, NP_SHARD, TOPK), np.float32)
    idxs = np.empty((T, NP_SHARD, TOPK), np.uint32)
    def _deint(a):
        # [128, NJ*TOPK] -> [T_CORE, TOPK]; target t = j*128 + p
        return (
            a.reshape(128, -1, TOPK).transpose(1, 0, 2).reshape(T_CORE, TOPK)
        )
    for c in range(N_CORES):
        px, ty = c % NP_SHARD, c // NP_SHARD
        vals[ty * T_CORE : (ty + 1) * T_CORE, px] = _deint(results[c]["cand_vals"])
        idxs[ty * T_CORE : (ty + 1) * T_CORE, px] = _deint(results[c]["cand_idx"])
    return _host_greedy(vals, idxs, pred, tgt)


# revision 11
# speedup vs baseline: 1.0446x; 1.0195x over previous
"""Trainium2 Bass kernel: greedy bbox-matching loss (nn_BboxLoss).

Full computation: L[t,p] = pairwise bbox loss (IoU / MSE mix), then greedy
per-target argmin over still-available preds, mean of selected losses.

Strategy (8 NeuronCores, preds sharded 8 x 1024):
  device: per core, compute negated-loss tiles [128 targets x 1024 preds]
          entirely in SBUF, then top-8 per target via max/max_index.
          - PE matmul (K=6) produces -mse directly in PSUM (coords cross-term
            plus folded -sq_p/4 and -sq_t/4 rows).
          - ACT computes the min(corner) terms via warm-table Relu chains.
          - DVE does the remaining elementwise passes + top-8 extraction.
  host:   merge the 8x8=64 candidates per target, run the (inherently
          sequential, O(T*64)) greedy walk with an exactness safety check;
          rare unsafe rows fall back to a full-row recompute.

Device returns v = iou (overlap) or -mse (no overlap); true loss = 1 - v.
"""
import numpy as np
from contextlib import ExitStack

P_TOTAL = 8192
T = 2048
N_CORES = 8
NP_SHARD = 4                  # pred shards
NT_SHARD = 2                  # target shards
P_CORE = P_TOTAL // NP_SHARD  # 2048 preds per core
T_CORE = T // NT_SHARD        # 1024 targets per core
NJ = T_CORE // 128            # 8 row tiles of 128 targets
EPS = 1e-7
TOPK = 8

_CACHE = {}


def _build_nc():
    import concourse.bacc as bacc
    import concourse.mybir as mybir
    from concourse.tile import TileContext

    f32 = mybir.dt.float32
    u32 = mybir.dt.uint32
    i32 = mybir.dt.int32
    Alu = mybir.AluOpType
    Act = mybir.ActivationFunctionType

    nc = bacc.Bacc()
    ps_d = nc.dram_tensor("pshard", [1, 5 * P_CORE], f32, kind="ExternalInput")
    hp_d = nc.dram_tensor("hpredT", [6, P_CORE], f32, kind="ExternalInput")
    tsc_d = nc.dram_tensor("tscal", [128, 5 * NJ], f32, kind="ExternalInput")
    tt_d = nc.dram_tensor("tgtT", [6, T_CORE], f32, kind="ExternalInput")
    vals_d = nc.dram_tensor("cand_vals", [128, NJ * TOPK], f32, kind="ExternalOutput")
    idx_d = nc.dram_tensor("cand_idx", [128, NJ * TOPK], u32, kind="ExternalOutput")

    with TileContext(nc) as tc, ExitStack() as ctx:
        const = ctx.enter_context(tc.tile_pool(name="const", bufs=1))
        work = ctx.enter_context(tc.tile_pool(name="work", bufs=2))
        psum = ctx.enter_context(
            tc.tile_pool(name="psum", bufs=2, space="PSUM")
        )

        HP = const.tile([6, P_CORE], f32)
        TSC = const.tile([128, 5, NJ], f32)
        TT6 = const.tile([6, T_CORE], f32)
        PLANES = const.tile([128, 5, P_CORE], f32)
        VALS = const.tile([128, NJ, TOPK], f32)
        IDX = const.tile([128, NJ, TOPK], u32)

        nc.sync.dma_start(HP[:], hp_d[:])
        nc.sync.dma_start(TSC[:].rearrange("p q j -> p (q j)"), tsc_d[:])
        nc.sync.dma_start(TT6[:], tt_d[:])

        # load the five per-pred rows into partition 0 of PLANES, then
        # replicate across all 128 partitions in-place, one plane at a time,
        # ordered by first use so compute can start early
        PLF = PLANES[:].rearrange("p q n -> p (q n)")
        nc.sync.dma_start(PLF[0:1, :], ps_d[:])
        for q in (1, 3, 0, 2, 4):   # X1P, Y1P, X0P, Y0P, AREAP
            nc.gpsimd.partition_broadcast(
                PLANES[:, q, :], PLANES[0:1, q, :]
            )

        X0P = PLANES[:, 0, :]
        X1P = PLANES[:, 1, :]
        Y0P = PLANES[:, 2, :]
        Y1P = PLANES[:, 3, :]
        AREAP = PLANES[:, 4, :]

        for j in range(NJ):
            x0t = TSC[:, 0, j : j + 1]
            x1t = TSC[:, 1, j : j + 1]
            y0t = TSC[:, 2, j : j + 1]
            y1t = TSC[:, 3, j : j + 1]
            ate = TSC[:, 4, j : j + 1]   # area_t + EPS

            # nl1 = cross/2 - sq_t/4 - sq_p/4 = -mse, straight out of PE
            nl1 = psum.tile([128, P_CORE], f32, tag="nl1")
            lhsT = TT6[:, j * 128 : (j + 1) * 128]
            for h in range(P_CORE // 512):
                nc.tensor.matmul(
                    nl1[:, h * 512 : (h + 1) * 512],
                    lhsT,
                    HP[:, h * 512 : (h + 1) * 512],
                    start=True,
                    stop=True,
                )

            ox1 = work.tile([128, P_CORE], f32, tag="ox1")
            oy1 = work.tile([128, P_CORE], f32, tag="oy1")
            ndx = work.tile([128, P_CORE], f32, tag="ndx")
            ndy = work.tile([128, P_CORE], f32, tag="ndy")
            inter = work.tile([128, P_CORE], f32, tag="inter")
            nmr = work.tile([128, P_CORE], f32, tag="nmr")
            ssum = work.tile([128, P_CORE], f32, tag="ssum")
            rcp = work.tile([128, P_CORE], f32, tag="rcp")
            fin = work.tile([128, P_CORE], f32, tag="fin")

            # ox1 = min(x1p, x1t) = relu(x1t - relu(x1t - x1p)); warm Relu table
            nc.scalar.activation(ox1[:], X1P, Act.Relu, bias=x1t, scale=-1.0)
            nc.scalar.activation(ox1[:], ox1[:], Act.Relu, bias=x1t, scale=-1.0)
            nc.scalar.activation(oy1[:], Y1P, Act.Relu, bias=y1t, scale=-1.0)
            nc.scalar.activation(oy1[:], oy1[:], Act.Relu, bias=y1t, scale=-1.0)

            # ndx = max(x0p, x0t) - ox1  (= -dx)
            nc.vector.scalar_tensor_tensor(
                ndx[:], X0P, x0t, ox1[:], op0=Alu.max, op1=Alu.subtract
            )
            nc.vector.scalar_tensor_tensor(
                ndy[:], Y0P, y0t, oy1[:], op0=Alu.max, op1=Alu.subtract
            )
            # nmr = relu(max(ndx, ndy)): bit pattern nonzero <=> no overlap
            nc.vector.scalar_tensor_tensor(
                nmr[:], ndx[:], 0.0, ndy[:], op0=Alu.max, op1=Alu.max
            )
            # inter = ndx*ndy (= dx*dy)
            nc.vector.tensor_tensor(inter[:], ndx[:], ndy[:], op=Alu.mult)
            # rank-equivalent transform: iou/(1+iou) = inter / S with
            # S = area_p + area_t + EPS > 0 (monotone in iou; sign preserved)
            nc.vector.tensor_scalar(ssum[:], AREAP, ate, None, op0=Alu.add)
            nc.vector.reciprocal(rcp[:], ssum[:])
            nc.vector.tensor_tensor(fin[:], inter[:], rcp[:], op=Alu.mult)
            # where no overlap, take -mse from PSUM
            nc.vector.copy_predicated(fin[:], nmr[:].bitcast(i32), nl1[:])

            nc.vector.max(out=VALS[:, j, :], in_=fin[:])
            nc.vector.max_index(IDX[:, j, :], VALS[:, j, :], fin[:])

        nc.sync.dma_start(vals_d[:], VALS[:].rearrange("p j k -> p (j k)"))
        nc.sync.dma_start(idx_d[:], IDX[:].rearrange("p j k -> p (j k)"))

    nc.compile()
    return nc


def _prep_core_inputs(pred, tgt):
    """Host-side O(P+T) derived quantities. pred [P,4], tgt [T,4] float32."""
    shared = {}
    x0t = tgt[:, 0] - tgt[:, 2] / 2
    x1t = tgt[:, 0] + tgt[:, 2] / 2
    y0t = tgt[:, 1] - tgt[:, 3] / 2
    y1t = tgt[:, 1] + tgt[:, 3] / 2
    ate = tgt[:, 2] * tgt[:, 3] + np.float32(EPS)
    qt4 = np.sum(tgt * tgt, axis=-1) / 4
    ones_t = np.ones_like(qt4)
    tscal = np.stack([x0t, x1t, y0t, y1t, ate]).astype(np.float32)  # [5, T]
    shared["tscal"] = tscal
    shared["tgtT"] = np.ascontiguousarray(
        np.concatenate([tgt.T, ones_t[None, :], -qt4[None, :]]).astype(np.float32)
    )

    in_maps = []
    for c in range(N_CORES):
        px = c % NP_SHARD
        sh = pred[px * P_CORE : (px + 1) * P_CORE]
        x0p = np.maximum(sh[:, 0] - sh[:, 2] / 2, np.float32(0.0))
        x1p = np.minimum(sh[:, 0] + sh[:, 2] / 2, np.float32(1.0))
        y0p = np.maximum(sh[:, 1] - sh[:, 3] / 2, np.float32(0.0))
        y1p = np.minimum(sh[:, 1] + sh[:, 3] / 2, np.float32(1.0))
        areap = sh[:, 2] * sh[:, 3]
        qp = np.sum(sh * sh, axis=-1) / 4
        ones_p = np.ones_like(qp)
        ty = c // NP_SHARD
        tsl = slice(ty * T_CORE, (ty + 1) * T_CORE)
        in_maps.append(
            {
                "pshard": np.ascontiguousarray(
                    np.stack([x0p, x1p, y0p, y1p, areap]).astype(np.float32)
                ).reshape(1, 5 * P_CORE),
                "hpredT": np.ascontiguousarray(
                    np.concatenate(
                        [0.5 * sh.T, -qp[None, :], ones_p[None, :]]
                    ).astype(np.float32)
                ),
                "tscal": np.ascontiguousarray(
                    shared["tscal"][:, tsl].reshape(5, -1, 128).transpose(2, 0, 1)
                    .reshape(128, -1)
                ),
                "tgtT": np.ascontiguousarray(shared["tgtT"][:, tsl]),
            }
        )
    return in_maps


def _row_loss(pred, trow):
    """Exact device-form loss of one target row vs all preds (numpy f32)."""
    x0p = np.maximum(pred[:, 0] - pred[:, 2] / 2, np.float32(0.0))
    x1p = np.minimum(pred[:, 0] + pred[:, 2] / 2, np.float32(1.0))
    y0p = np.maximum(pred[:, 1] - pred[:, 3] / 2, np.float32(0.0))
    y1p = np.minimum(pred[:, 1] + pred[:, 3] / 2, np.float32(1.0))
    areap = pred[:, 2] * pred[:, 3]
    x0t = trow[0] - trow[2] / 2
    x1t = trow[0] + trow[2] / 2
    y0t = trow[1] - trow[3] / 2
    y1t = trow[1] + trow[3] / 2
    ndx = np.maximum(x0p, x0t) - np.minimum(x1p, x1t)
    ndy = np.maximum(y0p, y0t) - np.minimum(y1p, y1t)
    inter = ndx * ndy
    nov = np.maximum(ndx, ndy) > 0
    dneg = (inter - (trow[2] * trow[3] + np.float32(EPS))) - areap
    with np.errstate(divide="ignore", invalid="ignore"):
        iou = (-inter) * np.reciprocal(dneg)
    cross = pred @ (0.5 * trow).astype(np.float32)
    nmse = (cross - np.sum(trow * trow) / 4) - np.sum(pred * pred, axis=-1) / 4
    v = np.where(nov, nmse, iou)  # device value; loss = 1 - v
    return (np.float32(1.0) - v).astype(np.float32)


def _pair_losses(p, t):
    """Reference-form loss for matched pairs p[i] <-> t[i] (numpy f32->f64)."""
    p = p.astype(np.float32); t = t.astype(np.float32)
    x0p = np.maximum(p[:, 0] - p[:, 2] / 2, np.float32(0.0))
    x1p = np.minimum(p[:, 0] + p[:, 2] / 2, np.float32(1.0))
    y0p = np.maximum(p[:, 1] - p[:, 3] / 2, np.float32(0.0))
    y1p = np.minimum(p[:, 1] + p[:, 3] / 2, np.float32(1.0))
    x0t = t[:, 0] - t[:, 2] / 2
    x1t = t[:, 0] + t[:, 2] / 2
    y0t = t[:, 1] - t[:, 3] / 2
    y1t = t[:, 1] + t[:, 3] / 2
    ox0 = np.maximum(x0t, x0p); ox1 = np.minimum(x1t, x1p)
    oy0 = np.maximum(y0t, y0p); oy1 = np.minimum(y1t, y1p)
    nov = (ox1 < ox0) | (oy1 < oy0)
    inter = (ox1 - ox0) * (oy1 - oy0)
    denom = p[:, 2] * p[:, 3] + t[:, 2] * t[:, 3] - inter + np.float32(EPS)
    iou = inter / denom
    mse = np.sum((p - t) * (p - t), axis=-1) / np.float32(4.0)
    return np.where(nov, np.float32(1.0) + mse,
                    np.float32(1.0) - iou).astype(np.float64)


def _host_greedy(vals, idxs, pred, tgt):
    """vals/idxs [T, NP_SHARD, TOPK]: per-target candidates from each pred shard."""
    NSH = NP_SHARD
    loss = (1.0 - vals.reshape(T, NSH * TOPK).astype(np.float64))
    gidx = (
        idxs.astype(np.int64)
        + (np.arange(NSH)[None, :, None] * P_CORE)
    ).reshape(T, NSH * TOPK)

    taken = np.zeros(P_TOTAL, dtype=bool)
    sel = np.zeros(T, dtype=np.int64)
    for t in range(T):
        lt, gt = loss[t], gidx[t]
        order = np.lexsort((gt, lt))
        chosen = -1
        depth = 0
        for d in order:
            if not taken[gt[d]]:
                chosen = d
                break
            depth += 1
        safe = chosen >= 0
        if safe and depth >= TOPK:
            # a fully-taken shard whose worst listed candidate is better than
            # our choice could hide the true argmin
            closs = lt[chosen]
            for s in range(NSH):
                blk = slice(s * TOPK, (s + 1) * TOPK)
                if lt[s * TOPK + TOPK - 1] < closs and taken[gt[blk]].all():
                    safe = False
                    break
        if safe:
            k = gt[chosen]
        else:
            row = _row_loss(pred, tgt[t]).astype(np.float64)
            row[taken] = np.inf
            k = int(np.argmin(row))
        taken[k] = True
        sel[t] = k
    # exact reference-form loss of the selected pairs
    return np.float32(_pair_losses(pred[sel], tgt).mean())


def kernel(pred_bboxes, target_bboxes):
    from concourse.bass_utils import run_bass_kernel_spmd

    pred = np.asarray(pred_bboxes, dtype=np.float32)[0]
    tgt = np.asarray(target_bboxes, dtype=np.float32)[0]

    if "nc" not in _CACHE:
        _CACHE["nc"] = _build_nc()
    nc = _CACHE["nc"]

    in_maps = _prep_core_inputs(pred, tgt)
    res = run_bass_kernel_spmd(nc, in_maps, list(range(N_CORES)))
    results = res.results
    # core c covers targets [ (c//NP) * T_CORE : ... ], pred shard c % NP
    vals = np.empty((T, NP_SHARD, TOPK), np.float32)
    idxs = np.empty((T, NP_SHARD, TOPK), np.uint32)
    def _deint(a):
        # [128, NJ*TOPK] -> [T_CORE, TOPK]; target t = j*128 + p
        return (
            a.reshape(128, -1, TOPK).transpose(1, 0, 2).reshape(T_CORE, TOPK)
        )
    for c in range(N_CORES):
        px, ty = c % NP_SHARD, c // NP_SHARD
        vals[ty * T_CORE : (ty + 1) * T_CORE, px] = _deint(results[c]["cand_vals"])
        idxs[ty * T_CORE : (ty + 1) * T_CORE, px] = _deint(results[c]["cand_idx"])
    return _host_greedy(vals, idxs, pred, tgt)


# revision 12
# speedup vs baseline: 1.2213x; 1.1692x over previous
"""Trainium2 Bass kernel: greedy bbox-matching loss (nn_BboxLoss).

Full computation: L[t,p] = pairwise bbox loss (IoU / MSE mix), then greedy
per-target argmin over still-available preds, mean of selected losses.

Strategy (8 NeuronCores, preds sharded 8 x 1024):
  device: per core, compute negated-loss tiles [128 targets x 1024 preds]
          entirely in SBUF, then top-8 per target via max/max_index.
          - PE matmul (K=6) produces -mse directly in PSUM (coords cross-term
            plus folded -sq_p/4 and -sq_t/4 rows).
          - ACT computes the min(corner) terms via warm-table Relu chains.
          - DVE does the remaining elementwise passes + top-8 extraction.
  host:   merge the 8x8=64 candidates per target, run the (inherently
          sequential, O(T*64)) greedy walk with an exactness safety check;
          rare unsafe rows fall back to a full-row recompute.

Device returns v = iou (overlap) or -mse (no overlap); true loss = 1 - v.
"""
import numpy as np
from contextlib import ExitStack

P_TOTAL = 8192
T = 2048
N_CORES = 8
NP_SHARD = 4                  # pred shards
NT_SHARD = 2                  # target shards
P_CORE = P_TOTAL // NP_SHARD  # 2048 preds per core
T_CORE = T // NT_SHARD        # 1024 targets per core
NJ = T_CORE // 128            # 8 row tiles of 128 targets
EPS = 1e-7
TOPK = 8

_CACHE = {}


def _build_nc():
    import concourse.bacc as bacc
    import concourse.mybir as mybir
    from concourse.tile import TileContext

    f32 = mybir.dt.float32
    u32 = mybir.dt.uint32
    i32 = mybir.dt.int32
    Alu = mybir.AluOpType
    Act = mybir.ActivationFunctionType

    nc = bacc.Bacc()
    ps_d = nc.dram_tensor("pshard", [1, 5 * P_CORE], f32, kind="ExternalInput")
    hp_d = nc.dram_tensor("hpredT", [6, P_CORE], f32, kind="ExternalInput")
    tsc_d = nc.dram_tensor("tscal", [128, 5 * NJ], f32, kind="ExternalInput")
    tt_d = nc.dram_tensor("tgtT", [6, T_CORE], f32, kind="ExternalInput")
    vals_d = nc.dram_tensor("cand_vals", [128, NJ * TOPK], f32, kind="ExternalOutput")
    idx_d = nc.dram_tensor("cand_idx", [128, NJ * TOPK], u32, kind="ExternalOutput")

    with TileContext(nc) as tc, ExitStack() as ctx:
        const = ctx.enter_context(tc.tile_pool(name="const", bufs=1))
        work = ctx.enter_context(tc.tile_pool(name="work", bufs=2))
        psum = ctx.enter_context(
            tc.tile_pool(name="psum", bufs=2, space="PSUM")
        )

        HP = const.tile([6, P_CORE], f32)
        TSC = const.tile([128, 5, NJ], f32)
        TT6 = const.tile([6, T_CORE], f32)
        PLANES = const.tile([128, 5, P_CORE], f32)
        VALS = const.tile([128, NJ, TOPK], f32)
        IDX = const.tile([128, NJ, TOPK], u32)

        nc.sync.dma_start(HP[:], hp_d[:])
        nc.sync.dma_start(TSC[:].rearrange("p q j -> p (q j)"), tsc_d[:])
        nc.sync.dma_start(TT6[:], tt_d[:])

        # load the five per-pred rows into partition 0 of PLANES, then
        # replicate across all 128 partitions in-place, one plane at a time,
        # ordered by first use so compute can start early
        TINY = const.tile([128, 1], f32)
        nc.vector.memset(TINY[:], 2e-38)
        PLF = PLANES[:].rearrange("p q n -> p (q n)")
        nc.sync.dma_start(PLF[0:1, :], ps_d[:])
        for q in (1, 3, 0, 2, 4):   # X1P, Y1P, X0P, Y0P, AREAP
            nc.gpsimd.partition_broadcast(
                PLANES[:, q, :], PLANES[0:1, q, :]
            )

        X0P = PLANES[:, 0, :]
        X1P = PLANES[:, 1, :]
        Y0P = PLANES[:, 2, :]
        Y1P = PLANES[:, 3, :]
        AREAP = PLANES[:, 4, :]

        for j in range(NJ):
            x0t = TSC[:, 0, j : j + 1]
            x1t = TSC[:, 1, j : j + 1]
            y0t = TSC[:, 2, j : j + 1]
            y1t = TSC[:, 3, j : j + 1]
            ate = TSC[:, 4, j : j + 1]   # area_t + EPS

            # nl1 = cross/2 - sq_t/4 - sq_p/4 = -mse, straight out of PE
            nl1 = psum.tile([128, P_CORE], f32, tag="nl1")
            lhsT = TT6[:, j * 128 : (j + 1) * 128]
            for h in range(P_CORE // 512):
                nc.tensor.matmul(
                    nl1[:, h * 512 : (h + 1) * 512],
                    lhsT,
                    HP[:, h * 512 : (h + 1) * 512],
                    start=True,
                    stop=True,
                )

            ox1 = work.tile([128, P_CORE], f32, tag="ox1")
            oy1 = work.tile([128, P_CORE], f32, tag="oy1")
            ndx = work.tile([128, P_CORE], f32, tag="ndx")
            ndy = work.tile([128, P_CORE], f32, tag="ndy")
            inter = work.tile([128, P_CORE], f32, tag="inter")
            nmr = work.tile([128, P_CORE], f32, tag="nmr")
            lnis = work.tile([128, P_CORE], f32, tag="lnis")
            lnS = work.tile([128, P_CORE], f32, tag="lnS")
            fin = work.tile([128, P_CORE], f32, tag="fin")

            # ox1 = min(x1p, x1t) = relu(x1t - relu(x1t - x1p)); warm Relu table
            nc.scalar.activation(ox1[:], X1P, Act.Relu, bias=x1t, scale=-1.0)
            nc.scalar.activation(ox1[:], ox1[:], Act.Relu, bias=x1t, scale=-1.0)
            nc.scalar.activation(oy1[:], Y1P, Act.Relu, bias=y1t, scale=-1.0)
            nc.scalar.activation(oy1[:], oy1[:], Act.Relu, bias=y1t, scale=-1.0)

            # ndx = max(x0p, x0t) - ox1  (= -dx)
            nc.vector.scalar_tensor_tensor(
                ndx[:], X0P, x0t, ox1[:], op0=Alu.max, op1=Alu.subtract
            )
            nc.vector.scalar_tensor_tensor(
                ndy[:], Y0P, y0t, oy1[:], op0=Alu.max, op1=Alu.subtract
            )
            # nmr = relu(max(ndx, ndy)): bit pattern nonzero <=> no overlap
            nc.vector.scalar_tensor_tensor(
                nmr[:], ndx[:], 0.0, ndy[:], op0=Alu.max, op1=Alu.max
            )
            # inter = ndx*ndy (= dx*dy)
            nc.vector.tensor_tensor(inter[:], ndx[:], ndy[:], op=Alu.mult)
            # log-domain rank key: ln(inter + 2e-38) - ln(area_p + area_t + EPS)
            # == monotone transform of iou (ln(iou/(1+iou))); both Ln and Relu
            # live in the natural_log act func set (one table, stays warm).
            # the -mse branch is shifted by -128 (folded into the matmul) so
            # every overlap key (>= ln(2e-38/2) ~ -88) outranks every
            # non-overlap key (<= -128); NaNs from negative inter only occur
            # at non-overlap positions, which copy_predicated overwrites.
            nc.scalar.activation(lnis[:], inter[:], Act.Ln, bias=TINY[:, 0:1])
            nc.scalar.activation(lnS[:], AREAP, Act.Ln, bias=ate)
            nc.vector.tensor_tensor(fin[:], lnis[:], lnS[:], op=Alu.subtract)
            # where no overlap, take -mse from PSUM
            nc.vector.copy_predicated(fin[:], nmr[:].bitcast(i32), nl1[:])

            nc.vector.max(out=VALS[:, j, :], in_=fin[:])
            nc.vector.max_index(IDX[:, j, :], VALS[:, j, :], fin[:])

        nc.sync.dma_start(vals_d[:], VALS[:].rearrange("p j k -> p (j k)"))
        nc.sync.dma_start(idx_d[:], IDX[:].rearrange("p j k -> p (j k)"))

    nc.compile()
    return nc


def _prep_core_inputs(pred, tgt):
    """Host-side O(P+T) derived quantities. pred [P,4], tgt [T,4] float32."""
    shared = {}
    x0t = tgt[:, 0] - tgt[:, 2] / 2
    x1t = tgt[:, 0] + tgt[:, 2] / 2
    y0t = tgt[:, 1] - tgt[:, 3] / 2
    y1t = tgt[:, 1] + tgt[:, 3] / 2
    ate = tgt[:, 2] * tgt[:, 3] + np.float32(EPS)
    qt4 = np.sum(tgt * tgt, axis=-1) / 4
    ones_t = np.ones_like(qt4)
    tscal = np.stack([x0t, x1t, y0t, y1t, ate]).astype(np.float32)  # [5, T]
    shared["tscal"] = tscal
    shared["tgtT"] = np.ascontiguousarray(
        np.concatenate(
            [tgt.T, ones_t[None, :], -(qt4 + np.float32(128.0))[None, :]]
        ).astype(np.float32)
    )

    in_maps = []
    for c in range(N_CORES):
        px = c % NP_SHARD
        sh = pred[px * P_CORE : (px + 1) * P_CORE]
        x0p = np.maximum(sh[:, 0] - sh[:, 2] / 2, np.float32(0.0))
        x1p = np.minimum(sh[:, 0] + sh[:, 2] / 2, np.float32(1.0))
        y0p = np.maximum(sh[:, 1] - sh[:, 3] / 2, np.float32(0.0))
        y1p = np.minimum(sh[:, 1] + sh[:, 3] / 2, np.float32(1.0))
        areap = sh[:, 2] * sh[:, 3]
        qp = np.sum(sh * sh, axis=-1) / 4
        ones_p = np.ones_like(qp)
        ty = c // NP_SHARD
        tsl = slice(ty * T_CORE, (ty + 1) * T_CORE)
        in_maps.append(
            {
                "pshard": np.ascontiguousarray(
                    np.stack([x0p, x1p, y0p, y1p, areap]).astype(np.float32)
                ).reshape(1, 5 * P_CORE),
                "hpredT": np.ascontiguousarray(
                    np.concatenate(
                        [0.5 * sh.T, -qp[None, :], ones_p[None, :]]
                    ).astype(np.float32)
                ),
                "tscal": np.ascontiguousarray(
                    shared["tscal"][:, tsl].reshape(5, -1, 128).transpose(2, 0, 1)
                    .reshape(128, -1)
                ),
                "tgtT": np.ascontiguousarray(shared["tgtT"][:, tsl]),
            }
        )
    return in_maps


def _row_loss(pred, trow):
    """Exact device-form loss of one target row vs all preds (numpy f32)."""
    x0p = np.maximum(pred[:, 0] - pred[:, 2] / 2, np.float32(0.0))
    x1p = np.minimum(pred[:, 0] + pred[:, 2] / 2, np.float32(1.0))
    y0p = np.maximum(pred[:, 1] - pred[:, 3] / 2, np.float32(0.0))
    y1p = np.minimum(pred[:, 1] + pred[:, 3] / 2, np.float32(1.0))
    areap = pred[:, 2] * pred[:, 3]
    x0t = trow[0] - trow[2] / 2
    x1t = trow[0] + trow[2] / 2
    y0t = trow[1] - trow[3] / 2
    y1t = trow[1] + trow[3] / 2
    ndx = np.maximum(x0p, x0t) - np.minimum(x1p, x1t)
    ndy = np.maximum(y0p, y0t) - np.minimum(y1p, y1t)
    inter = ndx * ndy
    nov = np.maximum(ndx, ndy) > 0
    dneg = (inter - (trow[2] * trow[3] + np.float32(EPS))) - areap
    with np.errstate(divide="ignore", invalid="ignore"):
        iou = (-inter) * np.reciprocal(dneg)
    cross = pred @ (0.5 * trow).astype(np.float32)
    nmse = (cross - np.sum(trow * trow) / 4) - np.sum(pred * pred, axis=-1) / 4
    v = np.where(nov, nmse, iou)  # device value; loss = 1 - v
    return (np.float32(1.0) - v).astype(np.float32)


def _pair_losses(p, t):
    """Reference-form loss for matched pairs p[i] <-> t[i] (numpy f32->f64)."""
    p = p.astype(np.float32); t = t.astype(np.float32)
    x0p = np.maximum(p[:, 0] - p[:, 2] / 2, np.float32(0.0))
    x1p = np.minimum(p[:, 0] + p[:, 2] / 2, np.float32(1.0))
    y0p = np.maximum(p[:, 1] - p[:, 3] / 2, np.float32(0.0))
    y1p = np.minimum(p[:, 1] + p[:, 3] / 2, np.float32(1.0))
    x0t = t[:, 0] - t[:, 2] / 2
    x1t = t[:, 0] + t[:, 2] / 2
    y0t = t[:, 1] - t[:, 3] / 2
    y1t = t[:, 1] + t[:, 3] / 2
    ox0 = np.maximum(x0t, x0p); ox1 = np.minimum(x1t, x1p)
    oy0 = np.maximum(y0t, y0p); oy1 = np.minimum(y1t, y1p)
    nov = (ox1 < ox0) | (oy1 < oy0)
    inter = (ox1 - ox0) * (oy1 - oy0)
    denom = p[:, 2] * p[:, 3] + t[:, 2] * t[:, 3] - inter + np.float32(EPS)
    iou = inter / denom
    mse = np.sum((p - t) * (p - t), axis=-1) / np.float32(4.0)
    return np.where(nov, np.float32(1.0) + mse,
                    np.float32(1.0) - iou).astype(np.float64)


def _host_greedy(vals, idxs, pred, tgt):
    """vals/idxs [T, NP_SHARD, TOPK]: per-target candidates from each pred shard."""
    NSH = NP_SHARD
    loss = (1.0 - vals.reshape(T, NSH * TOPK).astype(np.float64))
    gidx = (
        idxs.astype(np.int64)
        + (np.arange(NSH)[None, :, None] * P_CORE)
    ).reshape(T, NSH * TOPK)

    taken = np.zeros(P_TOTAL, dtype=bool)
    sel = np.zeros(T, dtype=np.int64)
    for t in range(T):
        lt, gt = loss[t], gidx[t]
        order = np.lexsort((gt, lt))
        chosen = -1
        depth = 0
        for d in order:
            if not taken[gt[d]]:
                chosen = d
                break
            depth += 1
        safe = chosen >= 0
        if safe and depth >= TOPK:
            # a fully-taken shard whose worst listed candidate is better than
            # our choice could hide the true argmin
            closs = lt[chosen]
            for s in range(NSH):
                blk = slice(s * TOPK, (s + 1) * TOPK)
                if lt[s * TOPK + TOPK - 1] < closs and taken[gt[blk]].all():
                    safe = False
                    break
        if safe:
            k = gt[chosen]
        else:
            row = _row_loss(pred, tgt[t]).astype(np.float64)
            row[taken] = np.inf
            k = int(np.argmin(row))
        taken[k] = True
        sel[t] = k
    # exact reference-form loss of the selected pairs
    return np.float32(_pair_losses(pred[sel], tgt).mean())


def kernel(pred_bboxes, target_bboxes):
    from concourse.bass_utils import run_bass_kernel_spmd

    pred = np.asarray(pred_bboxes, dtype=np.float32)[0]
    tgt = np.asarray(target_bboxes, dtype=np.float32)[0]

    if "nc" not in _CACHE:
        _CACHE["nc"] = _build_nc()
    nc = _CACHE["nc"]

    in_maps = _prep_core_inputs(pred, tgt)
    res = run_bass_kernel_spmd(nc, in_maps, list(range(N_CORES)))
    results = res.results
    # core c covers targets [ (c//NP) * T_CORE : ... ], pred shard c % NP
    vals = np.empty((T, NP_SHARD, TOPK), np.float32)
    idxs = np.empty((T, NP_SHARD, TOPK), np.uint32)
    def _deint(a):
        # [128, NJ*TOPK] -> [T_CORE, TOPK]; target t = j*128 + p
        return (
            a.reshape(128, -1, TOPK).transpose(1, 0, 2).reshape(T_CORE, TOPK)
        )
    for c in range(N_CORES):
        px, ty = c % NP_SHARD, c // NP_SHARD
        vals[ty * T_CORE : (ty + 1) * T_CORE, px] = _deint(results[c]["cand_vals"])
        idxs[ty * T_CORE : (ty + 1) * T_CORE, px] = _deint(results[c]["cand_idx"])
    return _host_greedy(vals, idxs, pred, tgt)


# revision 13
# speedup vs baseline: 1.2224x; 1.0008x over previous
"""Trainium2 Bass kernel: greedy bbox-matching loss (nn_BboxLoss).

Full computation: L[t,p] = pairwise bbox loss (IoU / MSE mix), then greedy
per-target argmin over still-available preds, mean of selected losses.

Strategy (8 NeuronCores, preds sharded 8 x 1024):
  device: per core, compute negated-loss tiles [128 targets x 1024 preds]
          entirely in SBUF, then top-8 per target via max/max_index.
          - PE matmul (K=6) produces -mse directly in PSUM (coords cross-term
            plus folded -sq_p/4 and -sq_t/4 rows).
          - ACT computes the min(corner) terms via warm-table Relu chains.
          - DVE does the remaining elementwise passes + top-8 extraction.
  host:   merge the 8x8=64 candidates per target, run the (inherently
          sequential, O(T*64)) greedy walk with an exactness safety check;
          rare unsafe rows fall back to a full-row recompute.

Device returns v = iou (overlap) or -mse (no overlap); true loss = 1 - v.
"""
import numpy as np
from contextlib import ExitStack

P_TOTAL = 8192
T = 2048
N_CORES = 8
NP_SHARD = 4                  # pred shards
NT_SHARD = 2                  # target shards
P_CORE = P_TOTAL // NP_SHARD  # 2048 preds per core
T_CORE = T // NT_SHARD        # 1024 targets per core
NJ = T_CORE // 128            # 8 row tiles of 128 targets
EPS = 1e-7
TOPK = 8

_CACHE = {}


def _build_nc():
    import concourse.bacc as bacc
    import concourse.mybir as mybir
    from concourse.tile import TileContext

    f32 = mybir.dt.float32
    u32 = mybir.dt.uint32
    i32 = mybir.dt.int32
    Alu = mybir.AluOpType
    Act = mybir.ActivationFunctionType

    nc = bacc.Bacc()
    ps_d = nc.dram_tensor("pshard", [1, 5 * P_CORE], f32, kind="ExternalInput")
    hp_d = nc.dram_tensor("hpredT", [6, P_CORE], f32, kind="ExternalInput")
    tsc_d = nc.dram_tensor("tscal", [128, 5 * NJ], f32, kind="ExternalInput")
    tt_d = nc.dram_tensor("tgtT", [6, T_CORE], f32, kind="ExternalInput")
    vals_d = nc.dram_tensor("cand_vals", [128, NJ * TOPK], f32, kind="ExternalOutput")
    idx_d = nc.dram_tensor("cand_idx", [128, NJ * TOPK], u32, kind="ExternalOutput")

    with TileContext(nc) as tc, ExitStack() as ctx:
        const = ctx.enter_context(tc.tile_pool(name="const", bufs=1))
        work = ctx.enter_context(tc.tile_pool(name="work", bufs=2))
        psum = ctx.enter_context(
            tc.tile_pool(name="psum", bufs=2, space="PSUM")
        )

        HP = const.tile([6, P_CORE], f32)
        TSC = const.tile([128, 5, NJ], f32)
        TT6 = const.tile([6, T_CORE], f32)
        PLANES = const.tile([128, 5, P_CORE], f32)
        VALS = const.tile([128, NJ, TOPK], f32)
        IDX = const.tile([128, NJ, TOPK], u32)

        nc.sync.dma_start(HP[:], hp_d[:])
        nc.sync.dma_start(TSC[:].rearrange("p q j -> p (q j)"), tsc_d[:])
        nc.sync.dma_start(TT6[:], tt_d[:])

        # load the five per-pred rows into partition 0 of PLANES, then
        # replicate across all 128 partitions in-place, one plane at a time,
        # ordered by first use so compute can start early
        TINY = const.tile([128, 1], f32)
        nc.vector.memset(TINY[:], 2e-38)
        PLF = PLANES[:].rearrange("p q n -> p (q n)")
        nc.sync.dma_start(PLF[0:1, :], ps_d[:])
        for q in (1, 3, 0, 2, 4):   # X1P, Y1P, X0P, Y0P, AREAP
            nc.gpsimd.partition_broadcast(
                PLANES[:, q, :], PLANES[0:1, q, :]
            )

        X0P = PLANES[:, 0, :]
        X1P = PLANES[:, 1, :]
        Y0P = PLANES[:, 2, :]
        Y1P = PLANES[:, 3, :]
        AREAP = PLANES[:, 4, :]

        for j in range(NJ):
            x0t = TSC[:, 0, j : j + 1]
            x1t = TSC[:, 1, j : j + 1]
            y0t = TSC[:, 2, j : j + 1]
            y1t = TSC[:, 3, j : j + 1]
            ate = TSC[:, 4, j : j + 1]   # area_t + EPS

            # nl1 = cross/2 - sq_t/4 - sq_p/4 = -mse, straight out of PE
            nl1 = psum.tile([128, P_CORE], f32, tag="nl1")
            lhsT = TT6[:, j * 128 : (j + 1) * 128]
            for h in range(P_CORE // 512):
                nc.tensor.matmul(
                    nl1[:, h * 512 : (h + 1) * 512],
                    lhsT,
                    HP[:, h * 512 : (h + 1) * 512],
                    start=True,
                    stop=True,
                )

            ox1 = work.tile([128, P_CORE], f32, tag="ox1")
            oy1 = work.tile([128, P_CORE], f32, tag="oy1")
            ndx = work.tile([128, P_CORE], f32, tag="ndx")
            ndy = work.tile([128, P_CORE], f32, tag="ndy")
            inter = work.tile([128, P_CORE], f32, tag="inter")
            nmr = work.tile([128, P_CORE], f32, tag="nmr")
            lnis = work.tile([128, P_CORE], f32, tag="lnis")
            lnS = work.tile([128, P_CORE], f32, tag="lnS")
            fin = work.tile([128, P_CORE], f32, tag="fin")

            # ox1 = min(x1p, x1t) = relu(x1t - relu(x1t - x1p)); warm Relu table
            nc.scalar.activation(ox1[:], X1P, Act.Relu, bias=x1t, scale=-1.0)
            nc.scalar.activation(ox1[:], ox1[:], Act.Relu, bias=x1t, scale=-1.0)
            nc.scalar.activation(oy1[:], Y1P, Act.Relu, bias=y1t, scale=-1.0)
            nc.scalar.activation(oy1[:], oy1[:], Act.Relu, bias=y1t, scale=-1.0)

            # ndx = max(x0p, x0t) - ox1  (= -dx)
            nc.vector.scalar_tensor_tensor(
                ndx[:], X0P, x0t, ox1[:], op0=Alu.max, op1=Alu.subtract
            )
            nc.vector.scalar_tensor_tensor(
                ndy[:], Y0P, y0t, oy1[:], op0=Alu.max, op1=Alu.subtract
            )
            # nmr = relu(max(ndx, ndy)): bit pattern nonzero <=> no overlap
            # inter = ndx*ndy (= dx*dy)
            nc.vector.tensor_tensor(inter[:], ndx[:], ndy[:], op=Alu.mult)
            nc.vector.scalar_tensor_tensor(
                nmr[:], ndx[:], 0.0, ndy[:], op0=Alu.max, op1=Alu.max
            )
            # log-domain rank key: ln(inter + 2e-38) - ln(area_p + area_t + EPS)
            # == monotone transform of iou (ln(iou/(1+iou))); both Ln and Relu
            # live in the natural_log act func set (one table, stays warm).
            # the -mse branch is shifted by -128 (folded into the matmul) so
            # every overlap key (>= ln(2e-38/2) ~ -88) outranks every
            # non-overlap key (<= -128); NaNs from negative inter only occur
            # at non-overlap positions, which copy_predicated overwrites.
            nc.scalar.activation(lnis[:], inter[:], Act.Ln, bias=TINY[:, 0:1])
            nc.scalar.activation(lnS[:], AREAP, Act.Ln, bias=ate)
            nc.vector.tensor_tensor(fin[:], lnis[:], lnS[:], op=Alu.subtract)
            # where no overlap, take -mse from PSUM
            nc.vector.copy_predicated(fin[:], nmr[:].bitcast(i32), nl1[:])

            nc.vector.max(out=VALS[:, j, :], in_=fin[:])
            nc.vector.max_index(IDX[:, j, :], VALS[:, j, :], fin[:])

        nc.sync.dma_start(vals_d[:], VALS[:].rearrange("p j k -> p (j k)"))
        nc.sync.dma_start(idx_d[:], IDX[:].rearrange("p j k -> p (j k)"))

    nc.compile()
    return nc


def _prep_core_inputs(pred, tgt):
    """Host-side O(P+T) derived quantities. pred [P,4], tgt [T,4] float32."""
    shared = {}
    x0t = tgt[:, 0] - tgt[:, 2] / 2
    x1t = tgt[:, 0] + tgt[:, 2] / 2
    y0t = tgt[:, 1] - tgt[:, 3] / 2
    y1t = tgt[:, 1] + tgt[:, 3] / 2
    ate = tgt[:, 2] * tgt[:, 3] + np.float32(EPS)
    qt4 = np.sum(tgt * tgt, axis=-1) / 4
    ones_t = np.ones_like(qt4)
    tscal = np.stack([x0t, x1t, y0t, y1t, ate]).astype(np.float32)  # [5, T]
    shared["tscal"] = tscal
    shared["tgtT"] = np.ascontiguousarray(
        np.concatenate(
            [tgt.T, ones_t[None, :], -(qt4 + np.float32(128.0))[None, :]]
        ).astype(np.float32)
    )

    in_maps = []
    for c in range(N_CORES):
        px = c % NP_SHARD
        sh = pred[px * P_CORE : (px + 1) * P_CORE]
        x0p = np.maximum(sh[:, 0] - sh[:, 2] / 2, np.float32(0.0))
        x1p = np.minimum(sh[:, 0] + sh[:, 2] / 2, np.float32(1.0))
        y0p = np.maximum(sh[:, 1] - sh[:, 3] / 2, np.float32(0.0))
        y1p = np.minimum(sh[:, 1] + sh[:, 3] / 2, np.float32(1.0))
        areap = sh[:, 2] * sh[:, 3]
        qp = np.sum(sh * sh, axis=-1) / 4
        ones_p = np.ones_like(qp)
        ty = c // NP_SHARD
        tsl = slice(ty * T_CORE, (ty + 1) * T_CORE)
        in_maps.append(
            {
                "pshard": np.ascontiguousarray(
                    np.stack([x0p, x1p, y0p, y1p, areap]).astype(np.float32)
                ).reshape(1, 5 * P_CORE),
                "hpredT": np.ascontiguousarray(
                    np.concatenate(
                        [0.5 * sh.T, -qp[None, :], ones_p[None, :]]
                    ).astype(np.float32)
                ),
                "tscal": np.ascontiguousarray(
                    shared["tscal"][:, tsl].reshape(5, -1, 128).transpose(2, 0, 1)
                    .reshape(128, -1)
                ),
                "tgtT": np.ascontiguousarray(shared["tgtT"][:, tsl]),
            }
        )
    return in_maps


def _row_loss(pred, trow):
    """Exact device-form loss of one target row vs all preds (numpy f32)."""
    x0p = np.maximum(pred[:, 0] - pred[:, 2] / 2, np.float32(0.0))
    x1p = np.minimum(pred[:, 0] + pred[:, 2] / 2, np.float32(1.0))
    y0p = np.maximum(pred[:, 1] - pred[:, 3] / 2, np.float32(0.0))
    y1p = np.minimum(pred[:, 1] + pred[:, 3] / 2, np.float32(1.0))
    areap = pred[:, 2] * pred[:, 3]
    x0t = trow[0] - trow[2] / 2
    x1t = trow[0] + trow[2] / 2
    y0t = trow[1] - trow[3] / 2
    y1t = trow[1] + trow[3] / 2
    ndx = np.maximum(x0p, x0t) - np.minimum(x1p, x1t)
    ndy = np.maximum(y0p, y0t) - np.minimum(y1p, y1t)
    inter = ndx * ndy
    nov = np.maximum(ndx, ndy) > 0
    dneg = (inter - (trow[2] * trow[3] + np.float32(EPS))) - areap
    with np.errstate(divide="ignore", invalid="ignore"):
        iou = (-inter) * np.reciprocal(dneg)
    cross = pred @ (0.5 * trow).astype(np.float32)
    nmse = (cross - np.sum(trow * trow) / 4) - np.sum(pred * pred, axis=-1) / 4
    v = np.where(nov, nmse, iou)  # device value; loss = 1 - v
    return (np.float32(1.0) - v).astype(np.float32)


def _pair_losses(p, t):
    """Reference-form loss for matched pairs p[i] <-> t[i] (numpy f32->f64)."""
    p = p.astype(np.float32); t = t.astype(np.float32)
    x0p = np.maximum(p[:, 0] - p[:, 2] / 2, np.float32(0.0))
    x1p = np.minimum(p[:, 0] + p[:, 2] / 2, np.float32(1.0))
    y0p = np.maximum(p[:, 1] - p[:, 3] / 2, np.float32(0.0))
    y1p = np.minimum(p[:, 1] + p[:, 3] / 2, np.float32(1.0))
    x0t = t[:, 0] - t[:, 2] / 2
    x1t = t[:, 0] + t[:, 2] / 2
    y0t = t[:, 1] - t[:, 3] / 2
    y1t = t[:, 1] + t[:, 3] / 2
    ox0 = np.maximum(x0t, x0p); ox1 = np.minimum(x1t, x1p)
    oy0 = np.maximum(y0t, y0p); oy1 = np.minimum(y1t, y1p)
    nov = (ox1 < ox0) | (oy1 < oy0)
    inter = (ox1 - ox0) * (oy1 - oy0)
    denom = p[:, 2] * p[:, 3] + t[:, 2] * t[:, 3] - inter + np.float32(EPS)
    iou = inter / denom
    mse = np.sum((p - t) * (p - t), axis=-1) / np.float32(4.0)
    return np.where(nov, np.float32(1.0) + mse,
                    np.float32(1.0) - iou).astype(np.float64)


def _host_greedy(vals, idxs, pred, tgt):
    """vals/idxs [T, NP_SHARD, TOPK]: per-target candidates from each pred shard."""
    NSH = NP_SHARD
    loss = (1.0 - vals.reshape(T, NSH * TOPK).astype(np.float64))
    gidx = (
        idxs.astype(np.int64)
        + (np.arange(NSH)[None, :, None] * P_CORE)
    ).reshape(T, NSH * TOPK)

    taken = np.zeros(P_TOTAL, dtype=bool)
    sel = np.zeros(T, dtype=np.int64)
    for t in range(T):
        lt, gt = loss[t], gidx[t]
        order = np.lexsort((gt, lt))
        chosen = -1
        depth = 0
        for d in order:
            if not taken[gt[d]]:
                chosen = d
                break
            depth += 1
        safe = chosen >= 0
        if safe and depth >= TOPK:
            # a fully-taken shard whose worst listed candidate is better than
            # our choice could hide the true argmin
            closs = lt[chosen]
            for s in range(NSH):
                blk = slice(s * TOPK, (s + 1) * TOPK)
                if lt[s * TOPK + TOPK - 1] < closs and taken[gt[blk]].all():
                    safe = False
                    break
        if safe:
            k = gt[chosen]
        else:
            row = _row_loss(pred, tgt[t]).astype(np.float64)
            row[taken] = np.inf
            k = int(np.argmin(row))
        taken[k] = True
        sel[t] = k
    # exact reference-form loss of the selected pairs
    return np.float32(_pair_losses(pred[sel], tgt).mean())


def kernel(pred_bboxes, target_bboxes):
    from concourse.bass_utils import run_bass_kernel_spmd

    pred = np.asarray(pred_bboxes, dtype=np.float32)[0]
    tgt = np.asarray(target_bboxes, dtype=np.float32)[0]

    if "nc" not in _CACHE:
        _CACHE["nc"] = _build_nc()
    nc = _CACHE["nc"]

    in_maps = _prep_core_inputs(pred, tgt)
    res = run_bass_kernel_spmd(nc, in_maps, list(range(N_CORES)))
    results = res.results
    # core c covers targets [ (c//NP) * T_CORE : ... ], pred shard c % NP
    vals = np.empty((T, NP_SHARD, TOPK), np.float32)
    idxs = np.empty((T, NP_SHARD, TOPK), np.uint32)
    def _deint(a):
        # [128, NJ*TOPK] -> [T_CORE, TOPK]; target t = j*128 + p
        return (
            a.reshape(128, -1, TOPK).transpose(1, 0, 2).reshape(T_CORE, TOPK)
        )
    for c in range(N_CORES):
        px, ty = c % NP_SHARD, c // NP_SHARD
        vals[ty * T_CORE : (ty + 1) * T_CORE, px] = _deint(results[c]["cand_vals"])
        idxs[ty * T_CORE : (ty + 1) * T_CORE, px] = _deint(results[c]["cand_idx"])
    return _host_greedy(vals, idxs, pred, tgt)
